# revision 1
# baseline (speedup 1.0000x reference)
"""Trainium2 Bass kernel for Detr3D cross-attention.

Sharding: query-parallel across 8 NeuronCores (128 queries per core).
Feature pyramids are replicated per core in a channel-last flat layout
(rows of 256 contiguous floats per spatial position), so the sparse
sampling stage is a per-camera indirect DMA gather (dma_gather) with
indices computed on-device from reference_points @ lidar2img.

Per-core device program:
  1. rpc = rp_h @ M^T via one PE matmul (queries on partitions).
  2. DVE chain computes sample coords and flat gather indices first
     (x and y fused into 48-wide tiles), folds them into dma_gather's
     wrapped int16 index layout using constant 0/1 "fold" matmuls on
     the PE, and launches the 6 per-camera gathers as early as
     possible (1024 rows x 2KB each; one row = a (query, level,
     y-tap); 512 floats cover the x0 and x0+1 taps at once).
  3. While the gathers stream, DVE computes bilinear weights, masks and
     sigmoid(attn) scaling, and the PE computes the positional-encoder
     branch.
  4. Per camera: DVE scales gathered rows by the combined weights and
     reduces the 16 (level,ytap,xtap) slots per query with a pairwise
     add tree, accumulating across cameras.
  5. Tail: W_out projection, residual adds, W_fin projection and final
     LayerNorm; each core emits its own (128, 64) output slice.

The host reassembles the 8 slices into the full (1024, 1, 64) output.
"""

import numpy as np

# ---------------------------------------------------------------- constants
Q, B, N, C = 1024, 1, 6, 256
NCORES = 8
QPC = Q // NCORES                       # 128 queries per core
LVL = [(116, 200), (58, 100), (29, 50), (15, 25)]
LV_BASE = [0, 23200, 29000, 30450]
CAM_ROWS = 30825                        # rows per camera (sum H*W)
FEAT_ROWS = N * CAM_ROWS + 135          # pad so 2KB reads never run off the end
IMG_H, IMG_W = 928.0, 1600.0
EPS = 1e-5
NPAIR = 24                              # (cam, level) pairs
MAGIC = 8388608.0                       # 2^23: round-to-nearest trick

_CACHE = {}


# ---------------------------------------------------------------- host prep
def _host_shared(inputs):
    """Inputs identical on every core."""
    feats = [inputs[f"feat{i}"] for i in range(4)]
    featT = np.zeros((FEAT_ROWS, C), np.float32)
    for c in range(N):
        for l, (H, W) in enumerate(LVL):
            r0 = c * CAM_ROWS + LV_BASE[l]
            featT[r0:r0 + H * W] = feats[l][0, c].reshape(C, H * W).T
    l2i = np.asarray(inputs["lidar2img"], np.float32)
    # mats[k, coord*6+cam] = l2i[0, cam, coord, k]   (coords x,y,z)
    mats = np.ascontiguousarray(
        np.transpose(l2i[0][:, 0:3, :], (2, 1, 0)).reshape(4, 18))

    def c24(fn):
        row = np.array([fn(lv) for cc in range(N) for lv in range(4)], np.float32)
        return np.ascontiguousarray(np.broadcast_to(row, (128, NPAIR)))

    def c48(fx, fy):
        row = np.array([f(lv) for f in (fx, fy) for cc in range(N) for lv in range(4)],
                       np.float32)
        return np.ascontiguousarray(np.broadcast_to(row, (128, 2 * NPAIR)))

    sxy_r = c48(lambda l: LVL[l][1] / IMG_W, lambda l: LVL[l][0] / IMG_H)
    wh_r = c48(lambda l: float(LVL[l][1]), lambda l: float(LVL[l][0]))
    whm1_r = c48(lambda l: float(LVL[l][1] - 1), lambda l: float(LVL[l][0] - 1))
    wt_r = c24(lambda l: float(LVL[l][1]))
    base_r = c24(lambda l: float(LV_BASE[l]))

    sfold = np.zeros((128, 1024), np.float32)
    for j in range(8):
        for p in range(16):
            sfold[16 * j + p, 128 * j + 16 * np.arange(8) + p] = 1.0
    i128 = np.eye(128, dtype=np.float32)
    i16x = np.ascontiguousarray(np.tile(i128, (1, 16)))   # (128, 2048)

    def repl(v, w):
        v = np.asarray(v, np.float32).reshape(1, w)
        return np.ascontiguousarray(np.broadcast_to(v, (128, w)))

    shared = dict(
        featT=featT, mats=mats,
        sxy_r=sxy_r, wh_r=wh_r, whm1_r=whm1_r, wt_r=wt_r, base_r=base_r,
        sfold=sfold, i128=i128, i16x=i16x,
        wqe=np.asarray(inputs["W_qe"], np.float32),
        wattn=np.asarray(inputs["W_attn"], np.float32),
        wout=np.asarray(inputs["W_out"], np.float32),
        pw1=np.asarray(inputs["pe_w1"], np.float32),
        pw2=np.asarray(inputs["pe_w2"], np.float32),
        wfin=np.asarray(inputs["W_fin"], np.float32),
        bqe_r=repl(inputs["b_qe"], 256),
        battn_r=repl(inputs["b_attn"], 24),
        bout_r=repl(inputs["b_out"], 256),
        pb1_r=repl(inputs["pe_b1"], 256),
        pg1_r=repl(inputs["pe_g1"], 256),
        pbe1_r=repl(inputs["pe_be1"], 256),
        pb2_r=repl(inputs["pe_b2"], 256),
        pg2_r=repl(inputs["pe_g2"], 256),
        pbe2_r=repl(inputs["pe_be2"], 256),
        bfin_r=repl(inputs["b_fin"], 64),
        gn_r=repl(inputs["g_norm"], 64),
        bn_r=repl(inputs["b_norm"], 64),
    )
    return shared


def _host_per_core(inputs, ci):
    qs, qe = ci * QPC, (ci + 1) * QPC
    qT = np.ascontiguousarray(np.asarray(inputs["query"], np.float32)[qs:qe, 0, :].T)
    qpT = np.ascontiguousarray(np.asarray(inputs["query_pos"], np.float32)[qs:qe, 0, :].T)
    rp = np.asarray(inputs["reference_points"], np.float32)[0, qs:qe, :]   # (128,3)
    rp_hT = np.concatenate([rp.T, np.ones((1, QPC), np.float32)], axis=0)  # (4,128)
    return dict(qT=qT, qpT=qpT, rp_hT=np.ascontiguousarray(rp_hT))


def make_in_maps(inputs):
    shared = _host_shared(inputs)
    return [dict(shared, **_host_per_core(inputs, ci)) for ci in range(NCORES)]


# ---------------------------------------------------------------- device
def _sub(t, off, dims):
    """Custom sub-AP of a pool tile: same partition dim, new free dims."""
    import concourse.bass as bass
    return bass.AP(t.tensor, t.offset + off, [list(t.ap[0])] + [list(d) for d in dims])


def build_nc():
    import concourse.bass as bass
    import concourse.bacc as bacc
    import concourse.mybir as mybir
    import concourse.tile as tile

    f32 = mybir.dt.float32
    i16 = mybir.dt.int16
    Alu = mybir.AluOpType
    Act = mybir.ActivationFunctionType

    nc = bacc.Bacc("TRN2", target_bir_lowering=False, debug=False,
                   enable_asserts=False, num_devices=NCORES)

    def din(name, shape):
        return nc.dram_tensor(name, list(shape), f32, kind="ExternalInput").ap()

    featT = din("featT", (FEAT_ROWS, C))
    mats = din("mats", (4, 18))
    rph_d = din("rp_hT", (4, 128))
    sxy_d, wh_d, whm1_d = din("sxy_r", (128, 48)), din("wh_r", (128, 48)), din("whm1_r", (128, 48))
    wt_d, base_d = din("wt_r", (128, 24)), din("base_r", (128, 24))
    sfold_d = din("sfold", (128, 1024))
    i128_d = din("i128", (128, 128))
    i16x_d = din("i16x", (128, 2048))
    qT_d, qpT_d = din("qT", (64, 128)), din("qpT", (64, 128))
    wqe_d, wattn_d = din("wqe", (64, 256)), din("wattn", (256, 24))
    wout_d, pw1_d = din("wout", (256, 256)), din("pw1", (3, 256))
    pw2_d, wfin_d = din("pw2", (256, 256)), din("wfin", (256, 64))
    bqe_d, battn_d = din("bqe_r", (128, 256)), din("battn_r", (128, 24))
    bout_d = din("bout_r", (128, 256))
    pb1_d, pg1_d, pbe1_d = din("pb1_r", (128, 256)), din("pg1_r", (128, 256)), din("pbe1_r", (128, 256))
    pb2_d, pg2_d, pbe2_d = din("pb2_r", (128, 256)), din("pg2_r", (128, 256)), din("pbe2_r", (128, 256))
    bfin_d, gn_d, bn_d = din("bfin_r", (128, 64)), din("gn_r", (128, 64)), din("bn_r", (128, 64))

    out_d = nc.dram_tensor("out", [QPC, 64], f32, kind="ExternalOutput").ap()

    from contextlib import ExitStack
    with tile.TileContext(nc) as tc, ExitStack() as stack:
        cp = stack.enter_context(tc.tile_pool(name="consts", bufs=1))
        wp = stack.enter_context(tc.tile_pool(name="work", bufs=1))
        gp = stack.enter_context(tc.tile_pool(name="gbuf", bufs=2))
        pp = stack.enter_context(tc.tile_pool(name="psum", bufs=4, space="PSUM"))

        def load(dram_ap, shape, name):
            t = cp.tile(shape, f32, name=name)
            nc.sync.dma_start(out=t[:, :], in_=dram_ap)
            return t

        def load2(dram_ap, shape, name):
            # tail-only constants go on the second HWDGE ring (ACT engine)
            t = cp.tile(shape, f32, name=name)
            nc.scalar.dma_start(out=t[:, :], in_=dram_ap)
            return t

        # chain-critical consts first
        mats_s = load(mats, (4, 18), "mats_s")
        rph_s = load(rph_d, (4, 128), "rph_s")
        sxy_s = load(sxy_d, (128, 48), "sxy_s")
        wh_s = load(wh_d, (128, 48), "wh_s")
        whm1_s = load(whm1_d, (128, 48), "whm1_s")
        wt_s = load(wt_d, (128, 24), "wt_s")
        base_s = load(base_d, (128, 24), "base_s")
        sfold_s = load(sfold_d, (128, 1024), "sfold_s")
        i128_s = load(i128_d, (128, 128), "i128_s")
        i16x_s = load2(i16x_d, (128, 2048), "i16x_s")
        qT_s = load(qT_d, (64, 128), "qT_s")
        qpT_s = load(qpT_d, (64, 128), "qpT_s")
        wqe_s = load(wqe_d, (64, 256), "wqe_s")
        wattn0 = load(wattn_d[0:128, :], (128, 24), "wattn0")
        wattn1 = load(wattn_d[128:256, :], (128, 24), "wattn1")
        wout0 = load2(wout_d[0:128, :], (128, 256), "wout0")
        wout1 = load2(wout_d[128:256, :], (128, 256), "wout1")
        pw1_s = load2(pw1_d, (3, 256), "pw1_s")
        pw2_0 = load2(pw2_d[0:128, :], (128, 256), "pw2_0")
        pw2_1 = load2(pw2_d[128:256, :], (128, 256), "pw2_1")
        wfin0 = load2(wfin_d[0:128, :], (128, 64), "wfin0")
        wfin1 = load2(wfin_d[128:256, :], (128, 64), "wfin1")
        bqe_s = load(bqe_d, (128, 256), "bqe_s")
        battn_s = load(battn_d, (128, 24), "battn_s")
        bout_s = load2(bout_d, (128, 256), "bout_s")
        pb1_s, pg1_s, pbe1_s = load2(pb1_d, (128, 256), "pb1_s"), load2(pg1_d, (128, 256), "pg1_s"), load(pbe1_d, (128, 256), "pbe1_s")
        pb2_s, pg2_s, pbe2_s = load2(pb2_d, (128, 256), "pb2_s"), load(pg2_d, (128, 256), "pg2_s"), load(pbe2_d, (128, 256), "pbe2_s")
        bfin_s, gn_s, bn_s = load2(bfin_d, (128, 64), "bfin_s"), load(gn_d, (128, 64), "gn_s"), load(bn_d, (128, 64), "bn_s")

        V = nc.vector
        S = nc.scalar
        T = nc.tensor
        GS = nc.gpsimd

        def vt(shape, name, dtype=f32, pool=wp, **kw):
            return pool.tile(list(shape), dtype, name=name, **kw)

        # ---------------- A: projection -----------------------------------
        rpc_p = pp.tile([128, 18], f32, name="rpc_p", tag="ps")
        T.matmul(rpc_p[:, :], lhsT=rph_s[:, :], rhs=mats_s[:, :], start=True, stop=True)
        RPC = vt((128, 18), "RPC")
        V.tensor_copy(out=RPC[:, :], in_=rpc_p[:, :])
        Xc, Yc, Zc = RPC[:, 0:6], RPC[:, 6:12], RPC[:, 12:18]

        zc = vt((128, 6), "zc")
        V.tensor_scalar_max(out=zc[:, :], in0=Zc, scalar1=EPS)
        rz = vt((128, 6), "rz")
        V.reciprocal(out=rz[:, :], in_=zc[:, :])
        XYq = vt((128, 12), "XYq")          # [x_img(6) | y_img(6)]
        V.tensor_tensor(out=XYq[:, 0:6], in0=Xc, in1=rz[:, :], op=Alu.mult)
        V.tensor_tensor(out=XYq[:, 6:12], in0=Yc, in1=rz[:, :], op=Alu.mult)

        # ---------------- B: index path (48-wide: [x(24) | y(24)]) --------
        def bc12(t):   # (128,12) -> (128,12,4) broadcast over levels
            return _sub(t, 0, [[1, 12], [0, 4]])

        def w3(t):     # (128,48) viewed as (128,12,4)
            return _sub(t, 0, [[4, 12], [1, 4]])

        xy = vt((128, 48), "xy")
        V.scalar_tensor_tensor(out=w3(xy), in0=bc12(XYq), scalar=1.0, in1=w3(sxy_s),
                               op0=Alu.mult, op1=Alu.mult)
        V.tensor_scalar_add(out=xy[:, :], in0=xy[:, :], scalar1=-0.5)

        t48 = vt((128, 48), "t48")
        V.tensor_scalar_add(out=t48[:, :], in0=xy[:, :], scalar1=1.0)
        fl48 = vt((128, 48), "fl48")
        V.tensor_scalar(out=fl48[:, :], in0=t48[:, :], scalar1=MAGIC, scalar2=MAGIC,
                        op0=Alu.add, op1=Alu.subtract)
        cr48 = vt((128, 48), "cr48")
        V.tensor_tensor(out=cr48[:, :], in0=fl48[:, :], in1=t48[:, :], op=Alu.is_gt)
        V.tensor_tensor(out=fl48[:, :], in0=fl48[:, :], in1=cr48[:, :], op=Alu.subtract)
        # fl48 = floor(xy + 1) = floor(xy) + 1
        ii0 = vt((128, 48), "ii0")          # [ix | iy0] clipped
        V.tensor_scalar(out=ii0[:, :], in0=fl48[:, :], scalar1=-1.0, scalar2=0.0,
                        op0=Alu.add, op1=Alu.max)
        V.tensor_tensor(out=ii0[:, :], in0=ii0[:, :], in1=whm1_s[:, :], op=Alu.min)
        iy1 = vt((128, 24), "iy1")          # clip(y0+1) = clip(fl48_y)
        V.tensor_scalar_max(out=iy1[:, :], in0=fl48[:, 24:48], scalar1=0.0)
        V.tensor_tensor(out=iy1[:, :], in0=iy1[:, :], in1=whm1_s[:, 24:48], op=Alu.min)

        ix, iy0 = ii0[:, 0:24], ii0[:, 24:48]
        fold_src = vt((128, 48), "fold_src")
        for yt, iy in ((0, iy0), (1, iy1[:, :])):
            dst = fold_src[:, 24 * yt:24 * yt + 24]
            V.tensor_tensor(out=dst, in0=iy, in1=wt_s[:, :], op=Alu.mult)
            V.tensor_tensor(out=dst, in0=dst, in1=ix, op=Alu.add)
            V.tensor_tensor(out=dst, in0=dst, in1=base_s[:, :], op=Alu.add)

        idx_p = pp.tile([128, 384], f32, name="idx_p", tag="psidx", bufs=1)
        for j in range(8):
            T.matmul(idx_p[:, 48 * j:48 * j + 48],
                     lhsT=sfold_s[:, 128 * j:128 * j + 128],
                     rhs=fold_src[:, :], start=True, stop=True)

        mega = vt((128, 384), "mega", dtype=i16)
        for yt in range(2):
            # dest col = 64c + 16lv + 8yt + j ; src col = 48j + 24yt + 4c + lv
            V.tensor_copy(
                out=_sub(mega, 8 * yt, [[64, 6], [16, 4], [1, 8]]),
                in_=_sub(idx_p, 24 * yt, [[4, 6], [1, 4], [48, 8]]))

        # ---------------- gathers (launch ASAP) ---------------------------
        g_tiles = []
        for cam in range(N):
            g_t = gp.tile([128, 4096], mybir.dt.float32r, name=f"g{cam}", tag="G", bufs=3)
            in_ap = bass.AP(featT.tensor, cam * CAM_ROWS * C,
                            [[C, CAM_ROWS + 130], [1, 512]]).bitcast(mybir.dt.float32r)
            GS.dma_gather(
                out_ap=_sub(g_t, 0, [[512, 8], [1, 512]]),
                in_ap=in_ap,
                idxs_ap=mega[:, 64 * cam:64 * cam + 64],
                num_idxs=1024, num_idxs_reg=1024,
                elem_size=512, elem_step=C)
            g_tiles.append(g_t)

        # ---------------- C: weights (overlap with gathers) ---------------
        v0 = vt((128, 48), "v0")
        tmp48 = vt((128, 48), "tmp48")
        V.tensor_scalar(out=v0[:, :], in0=xy[:, :], scalar1=0.0, scalar2=None, op0=Alu.is_ge)
        V.tensor_tensor(out=tmp48[:, :], in0=xy[:, :], in1=wh_s[:, :], op=Alu.is_lt)
        V.tensor_tensor(out=v0[:, :], in0=v0[:, :], in1=tmp48[:, :], op=Alu.mult)
        v1 = vt((128, 48), "v1")
        V.tensor_scalar(out=v1[:, :], in0=xy[:, :], scalar1=-1.0, scalar2=None, op0=Alu.is_ge)
        V.tensor_tensor(out=tmp48[:, :], in0=xy[:, :], in1=whm1_s[:, :], op=Alu.is_lt)
        V.tensor_tensor(out=v1[:, :], in0=v1[:, :], in1=tmp48[:, :], op=Alu.mult)
        sh = vt((128, 24), "sh")
        V.tensor_scalar(out=sh[:, :], in0=xy[:, 0:24], scalar1=0.0, scalar2=None, op0=Alu.is_lt)
        fr48 = vt((128, 48), "fr48")
        V.tensor_tensor(out=fr48[:, :], in0=t48[:, :], in1=fl48[:, :], op=Alu.subtract)
        w048 = vt((128, 48), "w048")
        V.tensor_scalar(out=w048[:, :], in0=fr48[:, :], scalar1=-1.0, scalar2=1.0,
                        op0=Alu.mult, op1=Alu.add)

        # mask per cam: front & inbounds (strict)
        front = vt((128, 6), "front")
        V.tensor_scalar(out=front[:, :], in0=Zc, scalar1=EPS, scalar2=None, op0=Alu.is_gt)
        m1 = vt((128, 12), "m1")
        m2 = vt((128, 12), "m2")
        V.tensor_scalar(out=m1[:, :], in0=XYq[:, :], scalar1=0.0, scalar2=None, op0=Alu.is_gt)
        V.tensor_scalar(out=m2[:, 0:6], in0=XYq[:, 0:6], scalar1=IMG_W, scalar2=None, op0=Alu.is_lt)
        V.tensor_scalar(out=m2[:, 6:12], in0=XYq[:, 6:12], scalar1=IMG_H, scalar2=None, op0=Alu.is_lt)
        V.tensor_tensor(out=m1[:, :], in0=m1[:, :], in1=m2[:, :], op=Alu.mult)
        mask = vt((128, 6), "mask")
        V.tensor_tensor(out=mask[:, :], in0=m1[:, 0:6], in1=m1[:, 6:12], op=Alu.mult)
        V.tensor_tensor(out=mask[:, :], in0=mask[:, :], in1=front[:, :], op=Alu.mult)

        # qe / attention
        qsT = vt((64, 128), "qsT")
        V.tensor_tensor(out=qsT[:, :], in0=qT_s[:, :], in1=qpT_s[:, :], op=Alu.add)
        qe_p = pp.tile([128, 256], f32, name="qe_p", tag="ps")
        T.matmul(qe_p[:, :], lhsT=qsT[:, :], rhs=wqe_s[:, :], start=True, stop=True)
        qe = vt((128, 256), "qe")
        V.scalar_tensor_tensor(out=qe[:, :], in0=qe_p[:, :], scalar=0.0, in1=bqe_s[:, :],
                               op0=Alu.add, op1=Alu.add)
        qeT0_p = pp.tile([128, 128], f32, name="qeT0_p", tag="ps")
        T.transpose(qeT0_p[:, :], qe[:, 0:128], i128_s[:, :])
        qeT1_p = pp.tile([128, 128], f32, name="qeT1_p", tag="ps")
        T.transpose(qeT1_p[:, :], qe[:, 128:256], i128_s[:, :])
        qeT0 = vt((128, 128), "qeT0")
        V.tensor_copy(out=qeT0[:, :], in_=qeT0_p[:, :])
        qeT1 = vt((128, 128), "qeT1")
        V.tensor_copy(out=qeT1[:, :], in_=qeT1_p[:, :])
        attw_p = pp.tile([128, 24], f32, name="attw_p", tag="ps")
        T.matmul(attw_p[:, :], lhsT=qeT0[:, :], rhs=wattn0[:, :], start=True, stop=False)
        T.matmul(attw_p[:, :], lhsT=qeT1[:, :], rhs=wattn1[:, :], start=False, stop=True)
        attwb = vt((128, 24), "attwb")
        V.scalar_tensor_tensor(out=attwb[:, :], in0=attw_p[:, :], scalar=0.0,
                               in1=battn_s[:, :], op0=Alu.add, op1=Alu.add)
        sgm = vt((128, 24), "sgm")
        S.activation(out=sgm[:, :], in_=attwb[:, :], func=Act.Sigmoid)
        s_eff = vt((128, 24), "s_eff")
        V.scalar_tensor_tensor(out=_sub(s_eff, 0, [[4, 6], [1, 4]]),
                               in0=_sub(mask, 0, [[1, 6], [0, 4]]), scalar=1.0,
                               in1=_sub(sgm, 0, [[4, 6], [1, 4]]),
                               op0=Alu.mult, op1=Alu.mult)

        # final per-slot weights: w_all col = 16c + 4lv + 2yt + half
        wlo = vt((128, 24), "wlo")
        whi = vt((128, 24), "whi")
        tb = vt((128, 24), "tb")
        V.tensor_tensor(out=wlo[:, :], in0=w048[:, 0:24], in1=v0[:, 0:24], op=Alu.mult)
        V.tensor_tensor(out=tb[:, :], in0=fr48[:, 0:24], in1=v1[:, 0:24], op=Alu.mult)
        V.tensor_tensor(out=whi[:, :], in0=tb[:, :], in1=sh[:, :], op=Alu.mult)
        V.tensor_tensor(out=wlo[:, :], in0=wlo[:, :], in1=whi[:, :], op=Alu.add)
        V.tensor_tensor(out=whi[:, :], in0=tb[:, :], in1=whi[:, :], op=Alu.subtract)
        wy0v = vt((128, 24), "wy0v")
        V.tensor_tensor(out=wy0v[:, :], in0=w048[:, 24:48], in1=v0[:, 24:48], op=Alu.mult)
        wy1v = vt((128, 24), "wy1v")
        V.tensor_tensor(out=wy1v[:, :], in0=fr48[:, 24:48], in1=v1[:, 24:48], op=Alu.mult)
        sy0 = vt((128, 24), "sy0")
        V.tensor_tensor(out=sy0[:, :], in0=s_eff[:, :], in1=wy0v[:, :], op=Alu.mult)
        sy1 = vt((128, 24), "sy1")
        V.tensor_tensor(out=sy1[:, :], in0=s_eff[:, :], in1=wy1v[:, :], op=Alu.mult)
        w_all = vt((128, 96), "w_all")
        for (syt, yt) in ((sy0, 0), (sy1, 1)):
            for (wx, half) in ((wlo, 0), (whi, 1)):
                V.tensor_tensor(
                    out=_sub(w_all, 2 * yt + half, [[16, 6], [4, 4]]),
                    in0=_sub(syt, 0, [[4, 6], [1, 4]]),
                    in1=_sub(wx, 0, [[4, 6], [1, 4]]), op=Alu.mult)

        # ---------------- helpers ----------------------------------------
        def transpose2(src, name):
            t0p = pp.tile([128, 128], f32, name=f"{name}0p", tag="ps")
            T.transpose(t0p[:, :], src[:, 0:128], i128_s[:, :])
            t1p = pp.tile([128, 128], f32, name=f"{name}1p", tag="ps")
            T.transpose(t1p[:, :], src[:, 128:256], i128_s[:, :])
            t0 = vt((128, 128), f"{name}0")
            V.tensor_copy(out=t0[:, :], in_=t0p[:, :])
            t1 = vt((128, 128), f"{name}1")
            V.tensor_copy(out=t1[:, :], in_=t1p[:, :])
            return t0, t1

        def layer_norm(x, g_s, b_s, dim, name):
            mu = vt((128, 1), f"{name}_mu")
            V.tensor_reduce(out=mu[:, :], in_=x[:, :], axis=mybir.AxisListType.X, op=Alu.add)
            V.tensor_scalar_mul(out=mu[:, :], in0=mu[:, :], scalar1=1.0 / dim)
            xm = vt((128, dim), f"{name}_xm")
            V.tensor_scalar(out=xm[:, :], in0=x[:, :], scalar1=mu[:, :], scalar2=None,
                            op0=Alu.subtract)
            sq = vt((128, dim), f"{name}_sq")
            vs = vt((128, 1), f"{name}_vs")
            V.scalar_tensor_tensor(out=sq[:, :], in0=xm[:, :], scalar=0.0, in1=xm[:, :],
                                   op0=Alu.add, op1=Alu.mult, accum_out=vs[:, :])
            std = vt((128, 1), f"{name}_std")
            V.tensor_scalar(out=std[:, :], in0=vs[:, :], scalar1=1.0 / dim,
                            scalar2=1e-5, op0=Alu.mult, op1=Alu.add)
            S.activation(out=std[:, :], in_=std[:, :], func=Act.Sqrt)
            rstd = vt((128, 1), f"{name}_rstd")
            V.reciprocal(out=rstd[:, :], in_=std[:, :])
            o = vt((128, dim), f"{name}_o")
            V.scalar_tensor_tensor(out=o[:, :], in0=xm[:, :], scalar=rstd[:, :],
                                   in1=g_s[:, :], op0=Alu.mult, op1=Alu.mult)
            V.tensor_tensor(out=o[:, :], in0=o[:, :], in1=b_s[:, :], op=Alu.add)
            return o

        # ---------------- D: positional branch (overlaps gathers) ---------
        pos1_p = pp.tile([128, 256], f32, name="pos1_p", tag="ps")
        T.matmul(pos1_p[:, :], lhsT=rph_s[0:3, :], rhs=pw1_s[:, :], start=True, stop=True)
        p1 = vt((128, 256), "p1")
        V.scalar_tensor_tensor(out=p1[:, :], in0=pos1_p[:, :], scalar=0.0,
                               in1=pb1_s[:, :], op0=Alu.add, op1=Alu.add)
        l1 = layer_norm(p1, pg1_s, pbe1_s, 256, "ln1")
        r1a = vt((128, 256), "r1a")
        S.activation(out=r1a[:, :], in_=l1[:, :], func=Act.Relu)
        rT0, rT1 = transpose2(r1a, "rT")
        pos2_p = pp.tile([128, 256], f32, name="pos2_p", tag="ps")
        T.matmul(pos2_p[:, :], lhsT=rT0[:, :], rhs=pw2_0[:, :], start=True, stop=False)
        T.matmul(pos2_p[:, :], lhsT=rT1[:, :], rhs=pw2_1[:, :], start=False, stop=True)
        p2 = vt((128, 256), "p2")
        V.scalar_tensor_tensor(out=p2[:, :], in0=pos2_p[:, :], scalar=0.0,
                               in1=pb2_s[:, :], op0=Alu.add, op1=Alu.add)
        l2 = layer_norm(p2, pg2_s, pbe2_s, 256, "ln2")
        pos = vt((128, 256), "pos")
        S.activation(out=pos[:, :], in_=l2[:, :], func=Act.Relu)

        # ---------------- E: per-camera scale + reduce on the PE ----------
        # psum_out += diag(w_all[:, 16c+rh]) @ G_rh for each (cam, slot):
        # applies per-(query,slot) weights and sums slots/cameras in PSUM.
        # float32r single-pass matmuls keep PE at 1 cycle/row; operands are
        # declared float32r so the BIR verifier sees rounded producers.
        f32r = mybir.dt.float32r
        psum_out = pp.tile([128, 256], f32, name="psum_out", tag="psout", bufs=1)
        for cam in range(N):
            g_t = g_tiles[cam]
            diag = gp.tile([128, 2048], f32r, name=f"diag{cam}", tag="diag", bufs=2)
            V.scalar_tensor_tensor(
                out=_sub(diag, 0, [[128, 16], [1, 128]]),
                in0=_sub(i16x_s, 0, [[128, 16], [1, 128]]),
                scalar=0.0,
                in1=_sub(w_all, 16 * cam, [[1, 16], [0, 128]]),
                op0=Alu.add, op1=Alu.mult)
            for rh in range(16):
                T.matmul(psum_out[:, :],
                         lhsT=diag[:, 128 * rh:128 * rh + 128],
                         rhs=g_t[:, 256 * rh:256 * rh + 256],
                         start=(cam == 0 and rh == 0),
                         stop=(cam == N - 1 and rh == 15))
        out_acc = vt((128, 256), "out_acc")
        V.tensor_copy(out=out_acc[:, :], in_=psum_out[:, :])
        oT0, oT1 = transpose2(out_acc, "oT")
        outw_p = pp.tile([128, 256], f32, name="outw_p", tag="ps")
        T.matmul(outw_p[:, :], lhsT=oT0[:, :], rhs=wout0[:, :], start=True, stop=False)
        T.matmul(outw_p[:, :], lhsT=oT1[:, :], rhs=wout1[:, :], start=False, stop=True)

        # ---------------- F: tail -----------------------------------------
        ssum = vt((128, 256), "ssum")
        V.scalar_tensor_tensor(out=ssum[:, :], in0=outw_p[:, :], scalar=0.0,
                               in1=bout_s[:, :], op0=Alu.add, op1=Alu.add)
        V.tensor_tensor(out=ssum[:, :], in0=ssum[:, :], in1=qe[:, :], op=Alu.add)
        V.tensor_tensor(out=ssum[:, :], in0=ssum[:, :], in1=pos[:, :], op=Alu.add)
        sT0, sT1 = transpose2(ssum, "sT")
        fin_p = pp.tile([128, 64], f32, name="fin_p", tag="ps")
        T.matmul(fin_p[:, :], lhsT=sT0[:, :], rhs=wfin0[:, :], start=True, stop=False)
        T.matmul(fin_p[:, :], lhsT=sT1[:, :], rhs=wfin1[:, :], start=False, stop=True)
        f1 = vt((128, 64), "f1")
        V.scalar_tensor_tensor(out=f1[:, :], in0=fin_p[:, :], scalar=0.0,
                               in1=bfin_s[:, :], op0=Alu.add, op1=Alu.add)
        fo = layer_norm(f1, gn_s, bn_s, 64, "ln3")
        nc.sync.dma_start(out=out_d, in_=fo[:, :])

    nc.compile()
    return nc


# ---------------------------------------------------------------- entry
def _ensure_ntff_hook():
    """Register the axon NTFF profiling hook if the image lacks antenv.axon_hooks."""
    import sys
    import types
    try:
        import antenv.axon_hooks  # noqa: F401
        return
    except ImportError:
        pass
    m = types.ModuleType("antenv.axon_hooks")
    _h = [None]
    m.set_axon_ntff_profile_hook = lambda h: _h.__setitem__(0, h)
    m.get_axon_ntff_profile_hook = lambda: _h[0]
    sys.modules["antenv.axon_hooks"] = m
    try:
        import antenv
        antenv.axon_hooks = m
    except ImportError:
        pass
    try:
        from trn_agent_boot.trn_boot import _ntff_profile_via_ctypes
        hook = _ntff_profile_via_ctypes("/opt/axon/libaxon_pjrt.so")
        if hook is not None:
            m.set_axon_ntff_profile_hook(hook)
    except Exception:
        pass


def kernel(**inputs):
    if "nc" not in _CACHE:
        _CACHE["nc"] = build_nc()
    nc = _CACHE["nc"]
    in_maps = make_in_maps(inputs)
    if _CACHE.get("trace"):
        _ensure_ntff_hook()
    from concourse.bass_utils import run_bass_kernel_spmd
    res = run_bass_kernel_spmd(nc, in_maps, core_ids=list(range(NCORES)),
                               trace=bool(_CACHE.get("trace")),
                               tmpdir=_CACHE.get("tmpdir"))
    _CACHE["last_results"] = res
    out = np.concatenate([res.results[ci]["out"] for ci in range(NCORES)], axis=0)
    return out.reshape(Q, B, 64).astype(np.float32)



# revision 3
# speedup vs baseline: 1.7131x; 1.7131x over previous
"""Trainium2 Bass kernel for Detr3D cross-attention (sparse compact gather).

Sharding: query-parallel across 8 NeuronCores. The host computes the
camera-projection geometry (mask, bilinear taps, gather indices) from the
tiny reference_points/lidar2img inputs, balances queries across cores by
valid-row count, and ships compact per-camera gather index lists. Only
~6% of the dense (query, cam, level, ytap) rows survive the mask, so the
per-core feature gather shrinks from 12.6 MB to ~0.9 MB.

Per-core device program:
  1. Per-camera dma_gather of the compact rows (512 floats each: both
     x-taps of one (query, level, ytap) sample) from the replicated
     channel-last feature pyramid in HBM.
  2. Meanwhile: qeT = [W_qe; b_qe]^T @ (q+q_pos) directly in transposed
     layout, attention logits attwT = W_attn^T @ qeT (+bias) -> sigmoid
     on the ACT engine as sigT (24, 128q); positional-encoder branch.
  3. Per 128-row block: a 24-row PE matmul selects sig[cl(r), q] rows,
     DVE folds it with the host-shipped bilinear weights into two
     one-hot scatter matrices diagA/diagB (query <- row), and the PE
     accumulates diag^T @ G into psum_out. Partial-partition matmuls
     ([0:R]) never touch unwritten gather rows.
  4. Tail: out @ (W_out@W_fin) + qe @ W_fin + pos @ W_fin + folded
     biases accumulate in one PSUM bank; final LayerNorm; DMA out.

The host reassembles / inverse-permutes the 8 (128, 64) slices.
"""

import numpy as np

# ---------------------------------------------------------------- constants
Q, B, N, C = 1024, 1, 6, 256
NCORES = 8
QPC = Q // NCORES                       # 128 queries per core
LVL = [(116, 200), (58, 100), (29, 50), (15, 25)]
LV_BASE = [0, 23200, 29000, 30450]
CAM_ROWS = 30825                        # rows per camera (sum H*W)
FEAT_ROWS = N * CAM_ROWS + 135          # pad so 2KB reads never run off the end
IMG_H, IMG_W = 928.0, 1600.0
EPS = 1e-5

_CACHE = {}


# ---------------------------------------------------------------- host geometry
def _geometry(inputs):
    """Compact row lists per (q, cam): (gather idx, lvl, bilA, bilB).

    Mirrors reference grid_sample exactly in f32. A row covers feature rows
    [idx, idx+1] (x-taps base, base+1) of one (q, cam, lvl, ytap) sample."""
    f32 = np.float32
    rp = np.asarray(inputs["reference_points"], f32)[0]
    l2i = np.asarray(inputs["lidar2img"], f32)[0]
    rph = np.concatenate([rp, np.ones((Q, 1), f32)], -1)
    rpc = np.einsum('nij,qj->nqi', l2i, rph).astype(f32)
    z = rpc[..., 2]
    front = z > EPS
    zc = np.maximum(z, EPS)
    xq = (rpc[..., 0] / zc).astype(f32)
    yq = (rpc[..., 1] / zc).astype(f32)
    gx = ((xq / f32(IMG_W) - f32(0.5)) * f32(2.0)).astype(f32)
    gy = ((yq / f32(IMG_H) - f32(0.5)) * f32(2.0)).astype(f32)
    mask = front & (gx > -1) & (gx < 1) & (gy > -1) & (gy < 1)

    rows = [[[] for _ in range(N)] for _ in range(Q)]
    for cam in range(N):
        for q in np.nonzero(mask[cam])[0]:
            for lvl, (H, W) in enumerate(LVL):
                x = ((gx[cam, q] + f32(1.0)) * f32(W) - f32(1.0)) * f32(0.5)
                y = ((gy[cam, q] + f32(1.0)) * f32(H) - f32(1.0)) * f32(0.5)
                x0 = int(np.floor(x))
                y0 = int(np.floor(y))
                fx = f32(x - x0)
                fy = f32(y - y0)
                wx = (f32(1.0) - fx, fx)
                wy = (f32(1.0) - fy, fy)
                for yt in range(2):
                    yy = y0 + yt
                    if not (0 <= yy < H):
                        continue
                    wyt = wy[yt]
                    vA = 0 <= x0 < W
                    vB = 0 <= x0 + 1 < W
                    if not (vA or vB):
                        continue
                    if vA:
                        base = x0
                        bilA = wyt * wx[0]
                        bilB = wyt * wx[1] if vB else f32(0.0)
                    else:
                        base = x0 + 1          # x0 == -1: first half IS the x1 tap
                        bilA = wyt * wx[1]
                        bilB = f32(0.0)
                    if bilA == 0.0 and bilB == 0.0:
                        continue
                    rows[q][cam].append(
                        (LV_BASE[lvl] + yy * W + base, lvl, f32(bilA), f32(bilB)))
    return rows


def _balance(rows):
    """Greedy query->core assignment minimizing per-cam max counts."""
    v = np.zeros((Q, N), np.int64)
    for q in range(Q):
        for c in range(N):
            v[q, c] = len(rows[q][c])
    order = np.argsort(-v.sum(1), kind="stable")
    cnt = np.zeros((NCORES, N), np.int64)
    nq = np.zeros(NCORES, np.int64)
    assign = np.zeros(Q, np.int64)
    for q in order:
        mx = cnt.max(0)
        best, bestcost = -1, None
        for ci in range(NCORES):
            if nq[ci] >= QPC:
                continue
            cost = np.maximum(cnt[ci] + v[q] - mx, 0).sum() * 1000 + cnt[ci].sum()
            if bestcost is None or cost < bestcost:
                best, bestcost = ci, cost
        cnt[best] += v[q]
        nq[best] += 1
        assign[q] = best
    qlists = [sorted(np.nonzero(assign == ci)[0].tolist()) for ci in range(NCORES)]
    return qlists, cnt


def _wrap_idx(idx, cap):
    """int16 wrapped layout: logical i -> partition i%16 (replicated x8), col i//16."""
    a = np.asarray(idx, np.int64)
    assert len(a) == cap and cap % 16 == 0
    w = a.reshape(cap // 16, 16).T                        # (16, cap//16)
    return np.ascontiguousarray(np.tile(w, (8, 1)).astype(np.int16))


def _prep(inputs):
    rows = _geometry(inputs)
    qlists, cnt = _balance(rows)
    caps = [int(min(-(-max(1, int(cnt[:, c].max())) // 16) * 16, 1024))
            if cnt[:, c].max() > 0 else 0 for c in range(N)]
    cams = [c for c in range(N) if caps[c] > 0]
    # block table: (cam, b, R) in gather/program order
    blocks = []
    for c in cams:
        nb = -(-caps[c] // 128)
        for b in range(nb):
            blocks.append((c, b, min(128, caps[c] - 128 * b)))
    NB = len(blocks)

    feats = [np.asarray(inputs[f"feat{i}"], np.float32) for i in range(4)]
    featT = np.zeros((FEAT_ROWS, C), np.float32)
    for c in range(N):
        for l, (H, W) in enumerate(LVL):
            r0 = c * CAM_ROWS + LV_BASE[l]
            featT[r0:r0 + H * W] = feats[l][0, c].reshape(C, H * W).T

    f32 = np.float32
    W_qe = np.asarray(inputs["W_qe"], f32)
    W_fin = np.asarray(inputs["W_fin"], f32)
    W_out = np.asarray(inputs["W_out"], f32)
    shared = dict(
        featT=featT,
        wqeb=np.ascontiguousarray(np.vstack([W_qe, np.asarray(inputs["b_qe"], f32)[None]])),
        wattn=np.asarray(inputs["W_attn"], f32),
        battn=np.ascontiguousarray(np.asarray(inputs["b_attn"], f32).reshape(24, 1)),
        pw1b=np.ascontiguousarray(np.vstack([np.asarray(inputs["pe_w1"], f32),
                                             np.asarray(inputs["pe_b1"], f32)[None]])),
        pw2=np.asarray(inputs["pe_w2"], f32),
        pb2r=np.ascontiguousarray(np.asarray(inputs["pe_b2"], f32).reshape(1, C)),
        wfin=W_fin,
        wof=np.ascontiguousarray((W_out @ W_fin).astype(f32)),
        bfinr=np.ascontiguousarray(
            (np.asarray(inputs["b_out"], f32) @ W_fin
             + np.asarray(inputs["b_fin"], f32)).reshape(1, 64)),
        ln4=np.ascontiguousarray(np.concatenate(
            [np.asarray(inputs[k], f32) for k in ("pe_g1", "pe_be1", "pe_g2", "pe_be2")]
        ).reshape(1, 4 * C)),
        gnbn=np.ascontiguousarray(np.concatenate(
            [np.asarray(inputs["g_norm"], f32), np.asarray(inputs["b_norm"], f32)]
        ).reshape(1, 128)),
    )

    query = np.asarray(inputs["query"], f32)[:, 0, :]
    query_pos = np.asarray(inputs["query_pos"], f32)[:, 0, :]
    rp = np.asarray(inputs["reference_points"], f32)[0]

    in_maps = []
    for ci in range(NCORES):
        ql = qlists[ci]
        d = dict(shared)
        d["qT"] = np.ascontiguousarray(query[ql].T)
        d["qpT"] = np.ascontiguousarray(query_pos[ql].T)
        d["rph"] = np.ascontiguousarray(
            np.vstack([rp[ql].T, np.ones((1, QPC), f32)]))
        meta1 = np.zeros((128, 3 * NB), f32)
        metaL = np.zeros((24, 128 * NB), f32)
        # per cam: compact row list in (qlocal asc) order, idx-0 padded to cap
        percam = {c: [] for c in cams}
        for qlocal, q in enumerate(ql):
            for c in cams:
                for (idx, lvl, bilA, bilB) in rows[q][c]:
                    percam[c].append((idx, qlocal, lvl, bilA, bilB))
        for j, (c, b, R) in enumerate(blocks):
            lst = percam[c]
            meta1[:, 3 * j] = -1.0
            for r in range(R):
                i = 128 * b + r
                if i < len(lst):
                    idx, qlocal, lvl, bilA, bilB = lst[i]
                    meta1[r, 3 * j] = qlocal
                    meta1[r, 3 * j + 1] = bilA
                    meta1[r, 3 * j + 2] = bilB
                    metaL[4 * c + lvl, 128 * j + r] = 1.0
        d["meta1"] = np.ascontiguousarray(meta1)
        d["metaL"] = np.ascontiguousarray(metaL)
        for c in cams:
            lst = percam[c]
            ilist = [e[0] for e in lst] + [0] * (caps[c] - len(lst))
            d[f"idx{c}"] = _wrap_idx(ilist, caps[c])
        in_maps.append(d)
    return in_maps, qlists, tuple(caps)


# ---------------------------------------------------------------- device
def _ap(t, off, nparts, dims):
    """Sub-AP of a pool tile: partition count nparts, custom free dims."""
    import concourse.bass as bass
    return bass.AP(t.tensor, t.offset + off,
                   [[t.ap[0][0], nparts]] + [list(d) for d in dims])


def build_nc(caps):
    import concourse.bass as bass
    import concourse.bacc as bacc
    import concourse.mybir as mybir
    import concourse.tile as tile

    f32 = mybir.dt.float32
    f32r = mybir.dt.float32r
    i16 = mybir.dt.int16
    i32 = mybir.dt.int32
    Alu = mybir.AluOpType
    Act = mybir.ActivationFunctionType

    cams = [c for c in range(N) if caps[c] > 0]
    blocks = []
    for c in cams:
        nb = -(-caps[c] // 128)
        for b in range(nb):
            blocks.append((c, b, min(128, caps[c] - 128 * b)))
    NB = len(blocks)

    nc = bacc.Bacc("TRN2", target_bir_lowering=False, debug=False,
                   enable_asserts=False, num_devices=NCORES)

    def din(name, shape, dtype=f32):
        return nc.dram_tensor(name, list(shape), dtype, kind="ExternalInput").ap()

    featT = din("featT", (FEAT_ROWS, C))
    wqeb_d = din("wqeb", (65, 256))
    wattn_d = din("wattn", (256, 24))
    battn_d = din("battn", (24, 1))
    pw1b_d = din("pw1b", (4, 256))
    pw2_d = din("pw2", (256, 256))
    pb2r_d = din("pb2r", (1, 256))
    wfin_d = din("wfin", (256, 64))
    wof_d = din("wof", (256, 64))
    bfinr_d = din("bfinr", (1, 64))
    ln4_d = din("ln4", (1, 1024))
    gnbn_d = din("gnbn", (1, 128))
    qT_d = din("qT", (64, 128))
    qpT_d = din("qpT", (64, 128))
    rph_d = din("rph", (4, 128))
    meta1_d = din("meta1", (128, 3 * NB))
    metaL_d = din("metaL", (24, 128 * NB))
    idx_d = {c: din(f"idx{c}", (128, caps[c] // 16), i16) for c in cams}

    out_d = nc.dram_tensor("out", [QPC, 64], f32, kind="ExternalOutput").ap()

    from contextlib import ExitStack
    with tile.TileContext(nc) as tc, ExitStack() as stack:
        cp = stack.enter_context(tc.tile_pool(name="consts", bufs=1))
        wp = stack.enter_context(tc.tile_pool(name="work", bufs=1))
        bp = stack.enter_context(tc.tile_pool(name="blk", bufs=3))
        gp = stack.enter_context(tc.tile_pool(name="gbuf", bufs=1))
        pp = stack.enter_context(tc.tile_pool(name="psum", bufs=4, space="PSUM"))

        V = nc.vector
        S = nc.scalar
        T = nc.tensor
        GS = nc.gpsimd

        def load(dram_ap, shape, name, dtype=f32):
            t = cp.tile(list(shape), dtype, name=name)
            nc.sync.dma_start(out=t[:, :], in_=dram_ap)
            return t

        def load2(dram_ap, shape, name):
            t = cp.tile(list(shape), f32, name=name)
            nc.scalar.dma_start(out=t[:, :], in_=dram_ap)
            return t

        # critical-path loads on the sync ring (indices first)
        idx_s = {c: load(idx_d[c], (128, caps[c] // 16), f"idx{c}_s", i16)
                 for c in cams}
        meta1_s = load(meta1_d, (128, 3 * NB), "meta1_s")
        metaL_s = load(metaL_d, (24, 128 * NB), "metaL_s")
        qT_s = load(qT_d, (64, 128), "qT_s")
        qpT_s = load(qpT_d, (64, 128), "qpT_s")
        wqeb_s = load(wqeb_d, (65, 256), "wqeb_s")
        wattn0 = load(wattn_d[0:128, :], (128, 24), "wattn0")
        wattn1 = load(wattn_d[128:256, :], (128, 24), "wattn1")
        battn_s = load(battn_d, (24, 1), "battn_s")
        rph_s = load(rph_d, (4, 128), "rph_s")
        # tail loads on the ACT ring
        pw1b_s = load2(pw1b_d, (4, 256), "pw1b_s")
        ln4_s = load2(ln4_d, (1, 1024), "ln4_s")
        gnbn_s = load2(gnbn_d, (1, 128), "gnbn_s")
        wfin0 = load2(wfin_d[0:128, :], (128, 64), "wfin0")
        wfin1 = load2(wfin_d[128:256, :], (128, 64), "wfin1")
        pw2_0 = load2(pw2_d[0:128, :], (128, 256), "pw2_0")
        pw2_1 = load2(pw2_d[128:256, :], (128, 256), "pw2_1")
        pb2r_s = load2(pb2r_d, (1, 256), "pb2r_s")
        wof0 = load2(wof_d[0:128, :], (128, 64), "wof0")
        wof1 = load2(wof_d[128:256, :], (128, 64), "wof1")
        bfinr_s = load2(bfinr_d, (1, 64), "bfinr_s")

        def vt(shape, name, dtype=f32, pool=wp, **kw):
            return pool.tile(list(shape), dtype, name=name, **kw)

        # ---------------- on-device iota / identity consts ----------------
        ioti = vt((128, 128), "ioti", dtype=i32)
        GS.iota(ioti[:, :], pattern=[[1, 128]], base=0, channel_multiplier=0)
        iota_f = vt((128, 128), "iota_f")
        V.tensor_copy(out=iota_f[:, :], in_=ioti[:, :])
        iotc = vt((128, 128), "iotc", dtype=i32)
        GS.iota(iotc[:, :], pattern=[[0, 128]], base=0, channel_multiplier=1)
        ident = vt((128, 128), "ident")
        V.tensor_tensor(out=ident[:, :], in0=ioti[:, :], in1=iotc[:, :],
                        op=Alu.is_equal)
        ones1 = vt((1, 128), "ones1")
        V.memset(ones1[:, :], 1.0)

        # ---------------- gathers (launch ASAP) ---------------------------
        g_t = {}
        for c in cams:
            nb = -(-caps[c] // 128)
            t = gp.tile([128, nb * 512], f32r, name=f"g{c}")
            in_ap = bass.AP(featT.tensor, c * CAM_ROWS * C,
                            [[C, CAM_ROWS + 130], [1, 512]]).bitcast(f32r)
            GS.dma_gather(
                out_ap=_ap(t, 0, 128, [[512, nb], [1, 512]]),
                in_ap=in_ap,
                idxs_ap=idx_s[c][:, :],
                num_idxs=caps[c], num_idxs_reg=caps[c],
                elem_size=512, elem_step=C)
            g_t[c] = t

        # ---------------- qeT / sigT --------------------------------------
        qs = vt((65, 128), "qs")
        V.tensor_tensor(out=qs[0:64, :], in0=qT_s[:, :], in1=qpT_s[:, :], op=Alu.add)
        V.memset(qs[64:65, :], 1.0)
        qeT0_p = pp.tile([128, 128], f32, name="qeT0_p", tag="ps")
        T.matmul(qeT0_p[:, :], lhsT=wqeb_s[:, 0:128], rhs=qs[:, :], start=True, stop=True)
        qeT1_p = pp.tile([128, 128], f32, name="qeT1_p", tag="ps")
        T.matmul(qeT1_p[:, :], lhsT=wqeb_s[:, 128:256], rhs=qs[:, :], start=True, stop=True)
        qeT0 = vt((128, 128), "qeT0")
        V.tensor_copy(out=qeT0[:, :], in_=qeT0_p[:, :])
        qeT1 = vt((128, 128), "qeT1")
        V.tensor_copy(out=qeT1[:, :], in_=qeT1_p[:, :])
        attwT_p = pp.tile([24, 128], f32, name="attwT_p", tag="ps")
        T.matmul(attwT_p[:, :], lhsT=wattn0[:, :], rhs=qeT0[:, :], start=True, stop=False)
        T.matmul(attwT_p[:, :], lhsT=wattn1[:, :], rhs=qeT1[:, :], start=False, stop=True)
        sigT = vt((24, 128), "sigT")
        S.activation(out=sigT[:, :], in_=attwT_p[:, :], func=Act.Sigmoid,
                     bias=battn_s[:, 0:1])

        # ---------------- fin accumulation chain (qe part, early) ---------
        fin_p = pp.tile([128, 64], f32, name="fin_p", tag="fin", bufs=1)
        T.matmul(fin_p[:, :], lhsT=qeT0[:, :], rhs=wfin0[:, :], start=True, stop=False)
        T.matmul(fin_p[:, :], lhsT=qeT1[:, :], rhs=wfin1[:, :], start=False, stop=False)
        T.matmul(fin_p[:, :], lhsT=ones1[:, :], rhs=bfinr_s[:, :], start=False, stop=False)

        # ---------------- helpers -----------------------------------------
        def transpose2(src, name, copy_eng=V):
            t0p = pp.tile([128, 128], f32, name=f"{name}0p", tag="ps")
            T.transpose(t0p[:, :], src[:, 0:128], ident[:, :])
            t1p = pp.tile([128, 128], f32, name=f"{name}1p", tag="ps")
            T.transpose(t1p[:, :], src[:, 128:256], ident[:, :])
            t0 = vt((128, 128), f"{name}0")
            t1 = vt((128, 128), f"{name}1")
            if copy_eng is S:
                S.activation(out=t0[:, :], in_=t0p[:, :], func=Act.Copy)
                S.activation(out=t1[:, :], in_=t1p[:, :], func=Act.Copy)
            else:
                V.tensor_copy(out=t0[:, :], in_=t0p[:, :])
                V.tensor_copy(out=t1[:, :], in_=t1p[:, :])
            return t0, t1

        def layer_norm(x, g_ap, b_ap, dim, name):
            mu = vt((128, 1), f"{name}_mu")
            V.tensor_reduce(out=mu[:, :], in_=x[:, :], axis=mybir.AxisListType.X, op=Alu.add)
            V.tensor_scalar_mul(out=mu[:, :], in0=mu[:, :], scalar1=1.0 / dim)
            xm = vt((128, dim), f"{name}_xm")
            V.tensor_scalar(out=xm[:, :], in0=x[:, :], scalar1=mu[:, :], scalar2=None,
                            op0=Alu.subtract)
            sq = vt((128, dim), f"{name}_sq")
            vs = vt((128, 1), f"{name}_vs")
            V.scalar_tensor_tensor(out=sq[:, :], in0=xm[:, :], scalar=0.0, in1=xm[:, :],
                                   op0=Alu.add, op1=Alu.mult, accum_out=vs[:, :])
            std = vt((128, 1), f"{name}_std")
            V.tensor_scalar(out=std[:, :], in0=vs[:, :], scalar1=1.0 / dim,
                            scalar2=1e-5, op0=Alu.mult, op1=Alu.add)
            S.activation(out=std[:, :], in_=std[:, :], func=Act.Sqrt)
            rstd = vt((128, 1), f"{name}_rstd")
            V.reciprocal(out=rstd[:, :], in_=std[:, :])
            o = vt((128, dim), f"{name}_o")
            V.scalar_tensor_tensor(out=o[:, :], in0=xm[:, :], scalar=rstd[:, :],
                                   in1=g_ap, op0=Alu.mult, op1=Alu.mult)
            V.tensor_tensor(out=o[:, :], in0=o[:, :], in1=b_ap, op=Alu.add)
            return o

        # ---------------- LN const broadcasts (ones-row matmuls) ----------
        ln_t = []
        for k in range(4):
            lnb_p = pp.tile([128, 256], f32, name=f"lnb{k}_p", tag="ps")
            T.matmul(lnb_p[:, :], lhsT=ones1[:, :], rhs=ln4_s[:, 256 * k:256 * k + 256],
                     start=True, stop=True)
            t = vt((128, 256), f"lnc{k}")
            S.activation(out=t[:, :], in_=lnb_p[:, :], func=Act.Copy)
            ln_t.append(t)
        pg1_s, pbe1_s, pg2_s, pbe2_s = ln_t
        gnbn_p = pp.tile([128, 128], f32, name="gnbn_p", tag="ps")
        T.matmul(gnbn_p[:, :], lhsT=ones1[:, :], rhs=gnbn_s[:, :], start=True, stop=True)
        gnbn_t = vt((128, 128), "gnbn_t")
        S.activation(out=gnbn_t[:, :], in_=gnbn_p[:, :], func=Act.Copy)

        # ---------------- positional branch -------------------------------
        pos1_p = pp.tile([128, 256], f32, name="pos1_p", tag="ps")
        T.matmul(pos1_p[:, :], lhsT=rph_s[:, :], rhs=pw1b_s[:, :], start=True, stop=True)
        p1 = vt((128, 256), "p1")
        V.tensor_copy(out=p1[:, :], in_=pos1_p[:, :])
        l1 = layer_norm(p1, pg1_s[:, :], pbe1_s[:, :], 256, "ln1")
        r1a = vt((128, 256), "r1a")
        S.activation(out=r1a[:, :], in_=l1[:, :], func=Act.Relu)
        rT0, rT1 = transpose2(r1a, "rT", copy_eng=S)
        pos2_p = pp.tile([128, 256], f32, name="pos2_p", tag="ps")
        T.matmul(pos2_p[:, :], lhsT=rT0[:, :], rhs=pw2_0[:, :], start=True, stop=False)
        T.matmul(pos2_p[:, :], lhsT=rT1[:, :], rhs=pw2_1[:, :], start=False, stop=False)
        T.matmul(pos2_p[:, :], lhsT=ones1[:, :], rhs=pb2r_s[:, :], start=False, stop=True)
        p2 = vt((128, 256), "p2")
        V.tensor_copy(out=p2[:, :], in_=pos2_p[:, :])
        l2 = layer_norm(p2, pg2_s[:, :], pbe2_s[:, :], 256, "ln2")
        pos = vt((128, 256), "pos")
        S.activation(out=pos[:, :], in_=l2[:, :], func=Act.Relu)
        posT0, posT1 = transpose2(pos, "posT", copy_eng=S)
        T.matmul(fin_p[:, :], lhsT=posT0[:, :], rhs=wfin0[:, :], start=False, stop=False)
        T.matmul(fin_p[:, :], lhsT=posT1[:, :], rhs=wfin1[:, :], start=False, stop=False)

        # ---------------- per-block scatter-weighted reduce ----------------
        psum_out = pp.tile([128, 256], f32, name="psum_out", tag="psout", bufs=1)
        for j, (c, b, R) in enumerate(blocks):
            onehot = bp.tile([128, 128], f32, name=f"oh{j}", tag="oh")
            GS.tensor_scalar(out=onehot[0:R, :], in0=iota_f[0:R, :],
                             scalar1=meta1_s[0:R, 3 * j:3 * j + 1], scalar2=None,
                             op0=Alu.is_equal)
            sigsel_p = pp.tile([128, 128], f32, name=f"sigsel{j}_p", tag="ps")
            T.matmul(sigsel_p[0:R, :], lhsT=metaL_s[:, 128 * j:128 * j + R],
                     rhs=sigT[:, :], start=True, stop=True)
            scratch = bp.tile([128, 128], f32, name=f"scr{j}", tag="scr")
            w_row = bp.tile([128, 1], f32, name=f"wr{j}", tag="wr")
            V.scalar_tensor_tensor(out=scratch[0:R, :], in0=sigsel_p[0:R, :],
                                   scalar=0.0, in1=onehot[0:R, :],
                                   op0=Alu.add, op1=Alu.mult,
                                   accum_out=w_row[0:R, :])
            wab = bp.tile([128, 2], f32, name=f"wab{j}", tag="wab")
            V.tensor_tensor(out=wab[0:R, :], in0=_ap(w_row, 0, R, [[0, 2]]),
                            in1=meta1_s[0:R, 3 * j + 1:3 * j + 3], op=Alu.mult)
            diagA = bp.tile([128, 128], f32r, name=f"dA{j}", tag="dA")
            V.tensor_scalar_mul(out=diagA[0:R, :], in0=onehot[0:R, :],
                                scalar1=wab[0:R, 0:1])
            diagB = bp.tile([128, 128], f32r, name=f"dB{j}", tag="dB")
            V.tensor_scalar_mul(out=diagB[0:R, :], in0=onehot[0:R, :],
                                scalar1=wab[0:R, 1:2])
            T.matmul(psum_out[:, :], lhsT=diagA[0:R, :],
                     rhs=g_t[c][0:R, 512 * b:512 * b + 256],
                     start=(j == 0), stop=False)
            T.matmul(psum_out[:, :], lhsT=diagB[0:R, :],
                     rhs=g_t[c][0:R, 512 * b + 256:512 * b + 512],
                     start=False, stop=(j == NB - 1))

        # ---------------- tail ---------------------------------------------
        out_acc = vt((128, 256), "out_acc")
        V.tensor_copy(out=out_acc[:, :], in_=psum_out[:, :])
        oT0, oT1 = transpose2(out_acc, "oT", copy_eng=V)
        T.matmul(fin_p[:, :], lhsT=oT0[:, :], rhs=wof0[:, :], start=False, stop=False)
        T.matmul(fin_p[:, :], lhsT=oT1[:, :], rhs=wof1[:, :], start=False, stop=True)
        f1 = vt((128, 64), "f1")
        V.tensor_copy(out=f1[:, :], in_=fin_p[:, :])
        fo = layer_norm(f1, gnbn_t[:, 0:64], gnbn_t[:, 64:128], 64, "ln3")
        nc.sync.dma_start(out=out_d, in_=fo[:, :])

    nc.compile()
    return nc


# ---------------------------------------------------------------- entry
def _ensure_ntff_hook():
    """Register the axon NTFF profiling hook if the image lacks antenv.axon_hooks."""
    import sys
    import types
    try:
        import antenv.axon_hooks  # noqa: F401
        return
    except ImportError:
        pass
    m = types.ModuleType("antenv.axon_hooks")
    _h = [None]
    m.set_axon_ntff_profile_hook = lambda h: _h.__setitem__(0, h)
    m.get_axon_ntff_profile_hook = lambda: _h[0]
    sys.modules["antenv.axon_hooks"] = m
    try:
        import antenv
        antenv.axon_hooks = m
    except ImportError:
        pass
    try:
        from trn_agent_boot.trn_boot import _ntff_profile_via_ctypes
        hook = _ntff_profile_via_ctypes("/opt/axon/libaxon_pjrt.so")
        if hook is not None:
            m.set_axon_ntff_profile_hook(hook)
    except Exception:
        pass


def kernel(**inputs):
    in_maps, qlists, caps = _prep(inputs)
    if _CACHE.get("caps") != caps:
        _CACHE["nc"] = build_nc(caps)
        _CACHE["caps"] = caps
    nc = _CACHE["nc"]
    if _CACHE.get("trace"):
        _ensure_ntff_hook()
    from concourse.bass_utils import run_bass_kernel_spmd
    res = run_bass_kernel_spmd(nc, in_maps, core_ids=list(range(NCORES)),
                               trace=bool(_CACHE.get("trace")),
                               tmpdir=_CACHE.get("tmpdir"))
    _CACHE["last_results"] = res
    out = np.zeros((Q, 64), np.float32)
    for ci in range(NCORES):
        out[qlists[ci]] = res.results[ci]["out"]
    return out.reshape(Q, B, 64).astype(np.float32)


# revision 13
# speedup vs baseline: 2.1632x; 1.2627x over previous
"""Trainium2 Bass kernel for Detr3D cross-attention (sparse compact gather).

Sharding: query-parallel across 8 NeuronCores. The host computes the
camera-projection geometry (mask, bilinear taps, gather indices) from the
tiny reference_points/lidar2img inputs, balances queries across cores by
valid-row count, and ships compact per-camera gather index lists. Only
~6% of the dense (query, cam, level, ytap) rows survive the mask, so the
per-core feature gather shrinks from 12.6 MB to ~0.9 MB.

Per-core device program (v3: f32r matmuls, batched diag build, packed
input DMAs, gather indices on the GpSimd DMA ring):
  1. Per-camera dma_gather of the compact rows (512 floats each: both
     x-taps of one (query, level, ytap) sample) from the replicated
     channel-last feature pyramid in HBM.
  2. Meanwhile: qeT = [W_qe; b_qe]^T @ (q+q_pos) directly in transposed
     layout, attwT = W_attn^T @ qeT -> sigmoid (ACT) as sigT (24, 128q);
     positional-encoder branch; W_fin contributions of qe accumulate in
     the fin PSUM bank.
  3. Per 128-row block j: a 24-row PE matmul selects sigsel[r, q] =
     sig[cl(r), q]; per 4-block chunk, three wide DVE ops build
     diag[r, 256j+128h+q] = (q == qidx[r]) * bil[r, h] * sigsel[r, q]
     via stride-0 broadcast views; PE accumulates diag^T @ G into
     psum_out with partial-partition matmuls ([0:R]).
  4. Tail: out @ (W_out@W_fin) rides the same fin PSUM bank as qe/pos;
     final LayerNorm straight out of PSUM; DMA out.

The host reassembles / inverse-permutes the 8 (128, 64) slices.
"""

import numpy as np

# ---------------------------------------------------------------- constants
Q, B, N, C = 1024, 1, 6, 256
NCORES = 8
QPC = Q // NCORES                       # 128 queries per core
LVL = [(116, 200), (58, 100), (29, 50), (15, 25)]
LV_BASE = [0, 23200, 29000, 30450]
CAM_ROWS = 30825                        # rows per camera (sum H*W)
FEAT_ROWS = N * CAM_ROWS + 135          # pad so 2KB reads never run off the end
IMG_H, IMG_W = 928.0, 1600.0
EPS = 1e-5

_CACHE = {}


# ---------------------------------------------------------------- host geometry
def _geometry(inputs):
    """Compact row lists per (q, cam): (gather idx, lvl, bilA, bilB).

    Mirrors reference grid_sample exactly in f32. A row covers feature rows
    [idx, idx+1] (x-taps base, base+1) of one (q, cam, lvl, ytap) sample."""
    f32 = np.float32
    rp = np.asarray(inputs["reference_points"], f32)[0]
    l2i = np.asarray(inputs["lidar2img"], f32)[0]
    rph = np.concatenate([rp, np.ones((Q, 1), f32)], -1)
    rpc = np.einsum('nij,qj->nqi', l2i, rph).astype(f32)
    z = rpc[..., 2]
    front = z > EPS
    zc = np.maximum(z, EPS)
    xq = (rpc[..., 0] / zc).astype(f32)
    yq = (rpc[..., 1] / zc).astype(f32)
    gx = ((xq / f32(IMG_W) - f32(0.5)) * f32(2.0)).astype(f32)
    gy = ((yq / f32(IMG_H) - f32(0.5)) * f32(2.0)).astype(f32)
    mask = front & (gx > -1) & (gx < 1) & (gy > -1) & (gy < 1)

    rows = [[[] for _ in range(N)] for _ in range(Q)]
    for cam in range(N):
        for q in np.nonzero(mask[cam])[0]:
            for lvl, (H, W) in enumerate(LVL):
                x = ((gx[cam, q] + f32(1.0)) * f32(W) - f32(1.0)) * f32(0.5)
                y = ((gy[cam, q] + f32(1.0)) * f32(H) - f32(1.0)) * f32(0.5)
                x0 = int(np.floor(x))
                y0 = int(np.floor(y))
                fx = f32(x - x0)
                fy = f32(y - y0)
                wx = (f32(1.0) - fx, fx)
                wy = (f32(1.0) - fy, fy)
                for yt in range(2):
                    yy = y0 + yt
                    if not (0 <= yy < H):
                        continue
                    wyt = wy[yt]
                    vA = 0 <= x0 < W
                    vB = 0 <= x0 + 1 < W
                    if not (vA or vB):
                        continue
                    if vA:
                        base = x0
                        bilA = wyt * wx[0]
                        bilB = wyt * wx[1] if vB else f32(0.0)
                    else:
                        base = x0 + 1          # x0 == -1: first half IS the x1 tap
                        bilA = wyt * wx[1]
                        bilB = f32(0.0)
                    if bilA == 0.0 and bilB == 0.0:
                        continue
                    rows[q][cam].append(
                        (LV_BASE[lvl] + yy * W + base, lvl, f32(bilA), f32(bilB)))
    return rows


def _balance(rows):
    """Greedy query->core assignment minimizing per-cam max counts."""
    v = np.zeros((Q, N), np.int64)
    for q in range(Q):
        for c in range(N):
            v[q, c] = len(rows[q][c])
    order = np.argsort(-v.sum(1), kind="stable")
    cnt = np.zeros((NCORES, N), np.int64)
    nq = np.zeros(NCORES, np.int64)
    assign = np.zeros(Q, np.int64)
    for q in order:
        mx = cnt.max(0)
        best, bestcost = -1, None
        for ci in range(NCORES):
            if nq[ci] >= QPC:
                continue
            cost = np.maximum(cnt[ci] + v[q] - mx, 0).sum() * 1000 + cnt[ci].sum()
            if bestcost is None or cost < bestcost:
                best, bestcost = ci, cost
        cnt[best] += v[q]
        nq[best] += 1
        assign[q] = best
    qlists = [sorted(np.nonzero(assign == ci)[0].tolist()) for ci in range(NCORES)]
    return qlists, cnt


def _wrap_idx(idx, cap):
    """int16 wrapped layout: logical i -> partition i%16 (replicated x8), col i//16."""
    a = np.asarray(idx, np.int64)
    assert len(a) == cap and cap % 16 == 0
    w = a.reshape(cap // 16, 16).T                        # (16, cap//16)
    return np.tile(w, (8, 1)).astype(np.int16)


def _prep(inputs):
    rows = _geometry(inputs)
    qlists, cnt = _balance(rows)
    caps = [int(min(-(-max(1, int(cnt[:, c].max())) // 16) * 16, 1024))
            if cnt[:, c].max() > 0 else 0 for c in range(N)]
    cams = [c for c in range(N) if caps[c] > 0]
    blocks = []
    for c in cams:
        nb = -(-caps[c] // 128)
        for b in range(nb):
            blocks.append((c, b, min(128, caps[c] - 128 * b)))
    NB = len(blocks)

    feats = [np.asarray(inputs[f"feat{i}"], np.float32) for i in range(4)]
    featT = np.zeros((FEAT_ROWS, C), np.float32)
    for c in range(N):
        for l, (H, W) in enumerate(LVL):
            r0 = c * CAM_ROWS + LV_BASE[l]
            featT[r0:r0 + H * W] = feats[l][0, c].reshape(C, H * W).T

    f32 = np.float32
    W_qe = np.asarray(inputs["W_qe"], f32)
    W_fin = np.asarray(inputs["W_fin"], f32)
    W_out = np.asarray(inputs["W_out"], f32)
    wattn = np.asarray(inputs["W_attn"], f32)
    pw2 = np.asarray(inputs["pe_w2"], f32)
    wof = (W_out @ W_fin).astype(f32)
    bfin_eff = (np.asarray(inputs["b_out"], f32) @ W_fin
                + np.asarray(inputs["b_fin"], f32))

    # pk1: [ln4(1024) | gnbn(128) | pb2r(256) | bfinr(64)]  -> (1, 1472)
    pk1 = np.concatenate([
        np.concatenate([np.asarray(inputs[k], f32)
                        for k in ("pe_g1", "pe_be1", "pe_g2", "pe_be2")]),
        np.asarray(inputs["g_norm"], f32), np.asarray(inputs["b_norm"], f32),
        np.asarray(inputs["pe_b2"], f32), bfin_eff,
    ]).reshape(1, 1472)
    # pk4: [rph placeholder(128) | pw1b(256)] per-core rph filled later
    pw1b = np.vstack([np.asarray(inputs["pe_w1"], f32),
                      np.asarray(inputs["pe_b1"], f32)[None]])     # (4, 256)
    # pkB (128, 768): [pw2_0|pw2_1|wfin0|wfin1|wof0|wof1]
    pkB = np.concatenate([pw2[0:128], pw2[128:256],
                          W_fin[0:128], W_fin[128:256],
                          wof[0:128], wof[128:256]], axis=1)
    wqeb = np.vstack([W_qe, np.asarray(inputs["b_qe"], f32)[None]])  # (65, 256)

    shared = dict(
        featT=featT,
        pk1=np.ascontiguousarray(pk1),
        pkB=np.ascontiguousarray(pkB),
        wqeb=np.ascontiguousarray(wqeb),
    )

    query = np.asarray(inputs["query"], f32)[:, 0, :]
    query_pos = np.asarray(inputs["query_pos"], f32)[:, 0, :]
    rp = np.asarray(inputs["reference_points"], f32)[0]
    battn = np.asarray(inputs["b_attn"], f32)

    idx_off = {}
    off = 0
    for c in cams:
        idx_off[c] = off
        off += caps[c] // 16
    IDXW = off

    in_maps = []
    for ci in range(NCORES):
        ql = qlists[ci]
        d = dict(shared)
        d["pk64"] = np.ascontiguousarray(
            np.concatenate([query[ql].T, query_pos[ql].T], axis=1))   # (64, 256)
        d["pk4"] = np.ascontiguousarray(np.concatenate(
            [np.vstack([rp[ql].T, np.ones((1, QPC), f32)]), pw1b], axis=1))  # (4, 384)
        # pkA (128, 24+24+3*NB): [wattn0|wattn1|qidx|bil]
        qidx = np.full((128, NB), -1.0, f32)
        bil = np.zeros((128, 2 * NB), f32)
        metaL = np.zeros((24, 128 * NB), f32)
        percam = {c: [] for c in cams}
        for qlocal, q in enumerate(ql):
            for c in cams:
                for (idx, lvl, bilA, bilB) in rows[q][c]:
                    percam[c].append((idx, qlocal, lvl, bilA, bilB))
        for j, (c, b, R) in enumerate(blocks):
            lst = percam[c]
            for r in range(R):
                i = 128 * b + r
                if i < len(lst):
                    idx, qlocal, lvl, bilA, bilB = lst[i]
                    qidx[r, j] = qlocal
                    bil[r, 2 * j] = bilA
                    bil[r, 2 * j + 1] = bilB
                    metaL[4 * c + lvl, 128 * j + r] = 1.0
        d["pkA"] = np.ascontiguousarray(
            np.concatenate([wattn[0:128], wattn[128:256], qidx, bil], axis=1))
        d["pk24"] = np.ascontiguousarray(
            np.concatenate([battn.reshape(24, 1), metaL], axis=1))   # (24, 1+128NB)
        pkidx = np.zeros((128, IDXW), np.int16)
        for c in cams:
            lst = percam[c]
            ilist = [e[0] for e in lst] + [0] * (caps[c] - len(lst))
            pkidx[:, idx_off[c]:idx_off[c] + caps[c] // 16] = _wrap_idx(ilist, caps[c])
        d["pkidx"] = np.ascontiguousarray(pkidx)
        in_maps.append(d)
    return in_maps, qlists, tuple(caps)


# ---------------------------------------------------------------- device
def _ap(t, off, nparts, dims):
    """Sub-AP of a pool tile: partition count nparts, custom free dims."""
    import concourse.bass as bass
    return bass.AP(t.tensor, t.offset + off,
                   [[t.ap[0][0], nparts]] + [list(d) for d in dims])


def build_nc(caps):
    import concourse.bass as bass
    import concourse.bacc as bacc
    import concourse.mybir as mybir
    import concourse.tile as tile

    f32 = mybir.dt.float32
    f32r = mybir.dt.float32r
    i16 = mybir.dt.int16
    i32 = mybir.dt.int32
    Alu = mybir.AluOpType
    Act = mybir.ActivationFunctionType

    cams = [c for c in range(N) if caps[c] > 0]
    blocks = []
    for c in cams:
        nb = -(-caps[c] // 128)
        for b in range(nb):
            blocks.append((c, b, min(128, caps[c] - 128 * b)))
    NB = len(blocks)
    idx_off = {}
    off = 0
    for c in cams:
        idx_off[c] = off
        off += caps[c] // 16
    IDXW = off
    CHUNKS = [list(range(s, min(s + 4, NB))) for s in range(0, NB, 4)]

    nc = bacc.Bacc("TRN2", target_bir_lowering=False, debug=False,
                   enable_asserts=False, num_devices=NCORES)

    def din(name, shape, dtype=f32):
        return nc.dram_tensor(name, list(shape), dtype, kind="ExternalInput").ap()

    featT = din("featT", (FEAT_ROWS, C))
    pk1_d = din("pk1", (1, 1472))
    pk4_d = din("pk4", (4, 384))
    pk64_d = din("pk64", (64, 256))
    wqeb_d = din("wqeb", (65, 256))
    pkA_d = din("pkA", (128, 48 + 3 * NB))
    pkB_d = din("pkB", (128, 768))
    pk24_d = din("pk24", (24, 1 + 128 * NB))
    pkidx_d = din("pkidx", (128, IDXW), i16)

    out_d = nc.dram_tensor("out", [QPC, 64], f32, kind="ExternalOutput").ap()

    from contextlib import ExitStack
    with tile.TileContext(nc) as tc, ExitStack() as stack:
        cp = stack.enter_context(tc.tile_pool(name="consts", bufs=1))
        wp = stack.enter_context(tc.tile_pool(name="work", bufs=1))
        gp = stack.enter_context(tc.tile_pool(name="gbuf", bufs=1))
        pp = stack.enter_context(tc.tile_pool(name="psum", bufs=4, space="PSUM"))

        V = nc.vector
        S = nc.scalar
        T = nc.tensor
        GS = nc.gpsimd

        def load(dram_ap, shape, name, dtype=f32r, eng=nc.sync):
            t = cp.tile(list(shape), dtype, name=name)
            if dtype in (f32r,):
                dram_ap = dram_ap.bitcast(f32r)
            eng.dma_start(out=t[:, :], in_=dram_ap)
            return t

        # gather indices on the GpSimd ring: gathers wait only on this DMA
        pkidx_s = load(pkidx_d, (128, IDXW), "pkidx_s", dtype=i16, eng=GS)
        # sync ring, in consumption order
        pk4_s = load(pk4_d, (4, 384), "pk4_s")
        pk1_s = load(pk1_d, (1, 1472), "pk1_s")
        pk64_s = load(pk64_d, (64, 256), "pk64_s")
        wqeb_s = load(wqeb_d, (65, 256), "wqeb_s")
        pkA_s = load(pkA_d, (128, 48 + 3 * NB), "pkA_s")
        pk24_s = load(pk24_d, (24, 1 + 128 * NB), "pk24_s")
        pkB_s = load(pkB_d, (128, 768), "pkB_s")
        qidx_c = pkA_s[:, 48:48 + NB]
        bil_c = pkA_s[:, 48 + NB:48 + 3 * NB]

        def vt(shape, name, dtype=f32r, pool=wp, **kw):
            return pool.tile(list(shape), dtype, name=name, **kw)

        # ---------------- on-device iota / identity consts ----------------
        ioti = vt((128, 128), "ioti", dtype=i32)
        GS.iota(ioti[:, :], pattern=[[1, 128]], base=0, channel_multiplier=0)
        iota_f = vt((128, 128), "iota_f", dtype=f32)
        V.tensor_copy(out=iota_f[:, :], in_=ioti[:, :])
        iotc = vt((128, 128), "iotc", dtype=i32)
        GS.iota(iotc[:, :], pattern=[[0, 128]], base=0, channel_multiplier=1)
        ident = vt((128, 128), "ident")
        V.tensor_tensor(out=ident[:, :], in0=ioti[:, :], in1=iotc[:, :],
                        op=Alu.is_equal)
        ones1f = vt((1, 128), "ones1f", dtype=f32)
        V.memset(ones1f[:, :], 1.0)
        ones1t = vt((1, 128), "ones1t")
        V.tensor_copy(out=ones1t[:, :], in_=ones1f[:, :])
        ones1 = ones1t[:, :]

        # ---------------- gathers (launch ASAP) ---------------------------
        g_t = {}
        for c in cams:
            nb = -(-caps[c] // 128)
            t = gp.tile([128, nb * 512], f32r, name=f"g{c}")
            in_ap = bass.AP(featT.tensor, c * CAM_ROWS * C,
                            [[C, CAM_ROWS + 130], [1, 512]]).bitcast(f32r)
            GS.dma_gather(
                out_ap=_ap(t, 0, 128, [[512, nb], [1, 512]]),
                in_ap=in_ap,
                idxs_ap=pkidx_s[:, idx_off[c]:idx_off[c] + caps[c] // 16],
                num_idxs=caps[c], num_idxs_reg=caps[c],
                elem_size=512, elem_step=C)
            g_t[c] = t

        # ---------------- qeT / sigT --------------------------------------
        qs = vt((65, 128), "qs")
        V.tensor_tensor(out=qs[0:64, :], in0=pk64_s[:, 0:128],
                        in1=pk64_s[:, 128:256], op=Alu.add)
        V.tensor_copy(out=qs[64:65, :], in_=ones1f[:, :])
        qs_r = qs[:, :]
        qeT0_p = pp.tile([128, 128], f32, name="qeT0_p", tag="ps")
        T.matmul(qeT0_p[:, :], lhsT=wqeb_s[:, 0:128], rhs=qs_r, start=True, stop=True)
        qeT1_p = pp.tile([128, 128], f32, name="qeT1_p", tag="ps")
        T.matmul(qeT1_p[:, :], lhsT=wqeb_s[:, 128:256], rhs=qs_r, start=True, stop=True)
        qeT0 = vt((128, 128), "qeT0")
        V.tensor_copy(out=qeT0[:, :], in_=qeT0_p[:, :])
        qeT1 = vt((128, 128), "qeT1")
        V.tensor_copy(out=qeT1[:, :], in_=qeT1_p[:, :])
        attwT_p = pp.tile([24, 128], f32, name="attwT_p", tag="ps")
        T.matmul(attwT_p[:, :], lhsT=pkA_s[:, 0:24], rhs=qeT0[:, :], start=True, stop=False)
        T.matmul(attwT_p[:, :], lhsT=pkA_s[:, 24:48], rhs=qeT1[:, :], start=False, stop=True)
        sigT = vt((24, 128), "sigT")
        S.activation(out=sigT[:, :], in_=attwT_p[:, :], func=Act.Sigmoid,
                     bias=pk24_s[:, 0:1])

        # ---------------- fin accumulation chain (qe part, early) ---------
        # pkB cols: [pw2_0(0:256) | pw2_1(256:512) | wfin0(512:576) |
        #            wfin1(576:640) | wof0(640:704) | wof1(704:768)]
        fin_p = pp.tile([128, 64], f32, name="fin_p", tag="fin", bufs=1)
        T.matmul(fin_p[:, :], lhsT=qeT0[:, :], rhs=pkB_s[:, 512:576], start=True, stop=False)
        T.matmul(fin_p[:, :], lhsT=qeT1[:, :], rhs=pkB_s[:, 576:640], start=False, stop=False)

        # ---------------- LN const broadcasts (ones-row matmuls) ----------
        lnbA_p = pp.tile([128, 512], f32, name="lnbA_p", tag="ps")
        T.matmul(lnbA_p[:, :], lhsT=ones1, rhs=pk1_s[:, 0:512], start=True, stop=True)
        lnbB_p = pp.tile([128, 512], f32, name="lnbB_p", tag="ps")
        T.matmul(lnbB_p[:, :], lhsT=ones1, rhs=pk1_s[:, 512:1024], start=True, stop=True)
        gnbn_p = pp.tile([128, 128], f32, name="gnbn_p", tag="ps")
        T.matmul(gnbn_p[:, :], lhsT=ones1, rhs=pk1_s[:, 1024:1152], start=True, stop=True)
        lncA = vt((128, 512), "lncA", dtype=f32)
        V.tensor_copy(out=lncA[:, :], in_=lnbA_p[:, :])
        lncB = vt((128, 512), "lncB", dtype=f32)
        V.tensor_copy(out=lncB[:, :], in_=lnbB_p[:, :])
        gnbn_t = vt((128, 128), "gnbn_t", dtype=f32)
        V.tensor_copy(out=gnbn_t[:, :], in_=gnbn_p[:, :])

        # ---------------- helpers -----------------------------------------
        def transpose2(src, name):
            t0p = pp.tile([128, 128], f32r, name=f"{name}0p", tag="ps")
            T.transpose(t0p[:, :], src[:, 0:128], ident[:, :])
            t1p = pp.tile([128, 128], f32r, name=f"{name}1p", tag="ps")
            T.transpose(t1p[:, :], src[:, 128:256], ident[:, :])
            t0 = vt((128, 128), f"{name}0")
            t1 = vt((128, 128), f"{name}1")
            V.tensor_copy(out=t0[:, :], in_=t0p[:, :])
            V.tensor_copy(out=t1[:, :], in_=t1p[:, :])
            return t0, t1

        def layer_norm(x_ap, g_ap, b_ap, dim, name):
            mu = vt((128, 1), f"{name}_mu", dtype=f32)
            V.tensor_reduce(out=mu[:, :], in_=x_ap, axis=mybir.AxisListType.X, op=Alu.add)
            V.tensor_scalar_mul(out=mu[:, :], in0=mu[:, :], scalar1=1.0 / dim)
            xm = vt((128, dim), f"{name}_xm", dtype=f32)
            V.tensor_scalar(out=xm[:, :], in0=x_ap, scalar1=mu[:, :], scalar2=None,
                            op0=Alu.subtract)
            sq = vt((128, dim), f"{name}_sq", dtype=f32)
            vs = vt((128, 1), f"{name}_vs", dtype=f32)
            V.scalar_tensor_tensor(out=sq[:, :], in0=xm[:, :], scalar=0.0, in1=xm[:, :],
                                   op0=Alu.add, op1=Alu.mult, accum_out=vs[:, :])
            std = vt((128, 1), f"{name}_std", dtype=f32)
            V.tensor_scalar(out=std[:, :], in0=vs[:, :], scalar1=1.0 / dim,
                            scalar2=1e-5, op0=Alu.mult, op1=Alu.add)
            S.activation(out=std[:, :], in_=std[:, :], func=Act.Sqrt)
            rstd = vt((128, 1), f"{name}_rstd", dtype=f32)
            V.reciprocal(out=rstd[:, :], in_=std[:, :])
            o = vt((128, dim), f"{name}_o", dtype=f32)
            V.scalar_tensor_tensor(out=o[:, :], in0=xm[:, :], scalar=rstd[:, :],
                                   in1=g_ap, op0=Alu.mult, op1=Alu.mult)
            V.tensor_tensor(out=o[:, :], in0=o[:, :], in1=b_ap, op=Alu.add)
            return o

        # ---------------- positional branch -------------------------------
        pos1_p = pp.tile([128, 256], f32, name="pos1_p", tag="ps")
        T.matmul(pos1_p[:, :], lhsT=pk4_s[:, 0:128], rhs=pk4_s[:, 128:384],
                 start=True, stop=True)
        l1 = layer_norm(pos1_p[:, :], lncA[:, 0:256], lncA[:, 256:512], 256, "ln1")
        r1a = vt((128, 256), "r1a")
        V.tensor_scalar_max(out=r1a[:, :], in0=l1[:, :], scalar1=0.0)
        rT0, rT1 = transpose2(r1a, "rT")
        pos2_p = pp.tile([128, 256], f32, name="pos2_p", tag="ps")
        T.matmul(pos2_p[:, :], lhsT=rT0[:, :], rhs=pkB_s[:, 0:256], start=True, stop=False)
        T.matmul(pos2_p[:, :], lhsT=rT1[:, :], rhs=pkB_s[:, 256:512], start=False, stop=False)
        T.matmul(pos2_p[:, :], lhsT=ones1, rhs=pk1_s[:, 1152:1408], start=False, stop=True)
        l2 = layer_norm(pos2_p[:, :], lncB[:, 0:256], lncB[:, 256:512], 256, "ln2")
        pos = vt((128, 256), "pos")
        V.tensor_scalar_max(out=pos[:, :], in0=l2[:, :], scalar1=0.0)
        posT0, posT1 = transpose2(pos, "posT")
        T.matmul(fin_p[:, :], lhsT=posT0[:, :], rhs=pkB_s[:, 512:576], start=False, stop=False)
        T.matmul(fin_p[:, :], lhsT=posT1[:, :], rhs=pkB_s[:, 576:640], start=False, stop=False)

        # ---------------- per-chunk scatter-weighted reduce ----------------
        psum_out = pp.tile([128, 256], f32, name="psum_out", tag="psout", bufs=1)
        sigsel_ps = []
        for ck, chunk in enumerate(CHUNKS):
            nb_ck = len(chunk)
            sp = pp.tile([128, 128 * nb_ck], f32, name=f"sigsel{ck}_p",
                         tag=f"sigsel{ck}", bufs=1)
            for jj, j in enumerate(chunk):
                c, b, R = blocks[j]
                T.matmul(sp[:, 128 * jj:128 * jj + 128],
                         lhsT=pk24_s[:, 1 + 128 * j:1 + 128 * j + 128],
                         rhs=sigT[:, :], start=True, stop=True)
            sigsel_ps.append(sp)
        for ck, chunk in enumerate(CHUNKS):
            nb_ck = len(chunk)
            j0 = chunk[0]
            sp = sigsel_ps[ck]
            ohb = vt((128, 256 * nb_ck), f"ohb{ck}", dtype=f32)
            V.tensor_tensor(
                out=ohb[:, :],
                in0=_ap(iota_f, 0, 128, [[0, nb_ck], [0, 2], [1, 128]]),
                in1=_ap(pkA_s, 48 + j0, 128, [[1, nb_ck], [0, 2], [0, 128]]),
                op=Alu.is_equal)
            V.tensor_tensor(
                out=ohb[:, :], in0=ohb[:, :],
                in1=_ap(pkA_s, 48 + NB + 2 * j0, 128, [[2, nb_ck], [1, 2], [0, 128]]),
                op=Alu.mult)
            diag = vt((128, 256 * nb_ck), f"diag{ck}")
            V.tensor_tensor(
                out=diag[:, :], in0=ohb[:, :],
                in1=_ap(sp, 0, 128, [[128, nb_ck], [0, 2], [1, 128]]),
                op=Alu.mult)
            for jj, j in enumerate(chunk):
                c, b, R = blocks[j]
                for h in range(2):
                    T.matmul(psum_out[:, :],
                             lhsT=diag[0:R, 256 * jj + 128 * h:256 * jj + 128 * h + 128],
                             rhs=g_t[c][0:R, 512 * b + 256 * h:512 * b + 256 * h + 256],
                             start=(j == 0 and h == 0), stop=(j == NB - 1 and h == 1))

        # ---------------- tail ---------------------------------------------
        out_acc = vt((128, 256), "out_acc")
        V.tensor_copy(out=out_acc[:, :], in_=psum_out[:, :])
        oT0, oT1 = transpose2(out_acc, "oT")
        T.matmul(fin_p[:, :], lhsT=oT0[:, :], rhs=pkB_s[:, 640:704], start=False, stop=False)
        T.matmul(fin_p[:, :], lhsT=oT1[:, :], rhs=pkB_s[:, 704:768], start=False, stop=False)
        T.matmul(fin_p[:, :], lhsT=ones1, rhs=pk1_s[:, 1408:1472], start=False, stop=True)
        fo = layer_norm(fin_p[:, :], gnbn_t[:, 0:64], gnbn_t[:, 64:128], 64, "ln3")
        nc.sync.dma_start(out=out_d, in_=fo[:, :])

    nc.compile()
    return nc


# ---------------------------------------------------------------- entry
def _ensure_ntff_hook():
    """Register the axon NTFF profiling hook if the image lacks antenv.axon_hooks."""
    import sys
    import types
    try:
        import antenv.axon_hooks  # noqa: F401
        return
    except ImportError:
        pass
    m = types.ModuleType("antenv.axon_hooks")
    _h = [None]
    m.set_axon_ntff_profile_hook = lambda h: _h.__setitem__(0, h)
    m.get_axon_ntff_profile_hook = lambda: _h[0]
    sys.modules["antenv.axon_hooks"] = m
    try:
        import antenv
        antenv.axon_hooks = m
    except ImportError:
        pass
    try:
        from trn_agent_boot.trn_boot import _ntff_profile_via_ctypes
        hook = _ntff_profile_via_ctypes("/opt/axon/libaxon_pjrt.so")
        if hook is not None:
            m.set_axon_ntff_profile_hook(hook)
    except Exception:
        pass


def kernel(**inputs):
    in_maps, qlists, caps = _prep(inputs)
    if _CACHE.get("caps") != caps:
        _CACHE["nc"] = build_nc(caps)
        _CACHE["caps"] = caps
    nc = _CACHE["nc"]
    if _CACHE.get("trace"):
        _ensure_ntff_hook()
    from concourse.bass_utils import run_bass_kernel_spmd
    res = run_bass_kernel_spmd(nc, in_maps, core_ids=list(range(NCORES)),
                               trace=bool(_CACHE.get("trace")),
                               tmpdir=_CACHE.get("tmpdir"))
    _CACHE["last_results"] = res
    out = np.zeros((Q, 64), np.float32)
    for ci in range(NCORES):
        out[qlists[ci]] = res.results[ci]["out"]
    return out.reshape(Q, B, 64).astype(np.float32)


# revision 14
# speedup vs baseline: 2.4687x; 1.1412x over previous
"""Trainium2 Bass kernel for Detr3D cross-attention (sparse compact gather).

Sharding: query-parallel across 8 NeuronCores. The host computes the
camera-projection geometry (mask, bilinear taps, gather indices) from the
tiny reference_points/lidar2img inputs, balances queries across cores by
valid-row count, and ships compact per-camera gather index lists. Only
~6% of the dense (query, cam, level, ytap) rows survive the mask, so the
per-core feature gather shrinks from 12.6 MB to ~0.9 MB.

Per-core device program (v3: f32r matmuls, batched diag build, packed
input DMAs, gather indices on the GpSimd DMA ring):
  1. Per-camera dma_gather of the compact rows (512 floats each: both
     x-taps of one (query, level, ytap) sample) from the replicated
     channel-last feature pyramid in HBM.
  2. Meanwhile: qeT = [W_qe; b_qe]^T @ (q+q_pos) directly in transposed
     layout, attwT = W_attn^T @ qeT -> sigmoid (ACT) as sigT (24, 128q);
     positional-encoder branch; W_fin contributions of qe accumulate in
     the fin PSUM bank.
  3. Per 128-row block j: a 24-row PE matmul selects sigsel[r, q] =
     sig[cl(r), q]; per 4-block chunk, three wide DVE ops build
     diag[r, 256j+128h+q] = (q == qidx[r]) * bil[r, h] * sigsel[r, q]
     via stride-0 broadcast views; PE accumulates diag^T @ G into
     psum_out with partial-partition matmuls ([0:R]).
  4. Tail: out @ (W_out@W_fin) rides the same fin PSUM bank as qe/pos;
     final LayerNorm straight out of PSUM; DMA out.

The host reassembles / inverse-permutes the 8 (128, 64) slices.
"""

import numpy as np

# ---------------------------------------------------------------- constants
Q, B, N, C = 1024, 1, 6, 256
NCORES = 8
QPC = Q // NCORES                       # 128 queries per core
LVL = [(116, 200), (58, 100), (29, 50), (15, 25)]
LV_BASE = [0, 23200, 29000, 30450]
CAM_ROWS = 30825                        # rows per camera (sum H*W)
FEAT_ROWS = N * CAM_ROWS + 135          # pad so 2KB reads never run off the end
IMG_H, IMG_W = 928.0, 1600.0
EPS = 1e-5

_CACHE = {}


# ---------------------------------------------------------------- host geometry
def _geometry(inputs):
    """Compact row lists per (q, cam): (gather idx, lvl, bilA, bilB).

    Mirrors reference grid_sample exactly in f32. A row covers feature rows
    [idx, idx+1] (x-taps base, base+1) of one (q, cam, lvl, ytap) sample."""
    f32 = np.float32
    rp = np.asarray(inputs["reference_points"], f32)[0]
    l2i = np.asarray(inputs["lidar2img"], f32)[0]
    rph = np.concatenate([rp, np.ones((Q, 1), f32)], -1)
    rpc = np.einsum('nij,qj->nqi', l2i, rph).astype(f32)
    z = rpc[..., 2]
    front = z > EPS
    zc = np.maximum(z, EPS)
    xq = (rpc[..., 0] / zc).astype(f32)
    yq = (rpc[..., 1] / zc).astype(f32)
    gx = ((xq / f32(IMG_W) - f32(0.5)) * f32(2.0)).astype(f32)
    gy = ((yq / f32(IMG_H) - f32(0.5)) * f32(2.0)).astype(f32)
    mask = front & (gx > -1) & (gx < 1) & (gy > -1) & (gy < 1)

    rows = [[[] for _ in range(N)] for _ in range(Q)]
    for cam in range(N):
        for q in np.nonzero(mask[cam])[0]:
            for lvl, (H, W) in enumerate(LVL):
                x = ((gx[cam, q] + f32(1.0)) * f32(W) - f32(1.0)) * f32(0.5)
                y = ((gy[cam, q] + f32(1.0)) * f32(H) - f32(1.0)) * f32(0.5)
                x0 = int(np.floor(x))
                y0 = int(np.floor(y))
                fx = f32(x - x0)
                fy = f32(y - y0)
                wx = (f32(1.0) - fx, fx)
                wy = (f32(1.0) - fy, fy)
                for yt in range(2):
                    yy = y0 + yt
                    if not (0 <= yy < H):
                        continue
                    wyt = wy[yt]
                    vA = 0 <= x0 < W
                    vB = 0 <= x0 + 1 < W
                    if not (vA or vB):
                        continue
                    if vA:
                        base = x0
                        bilA = wyt * wx[0]
                        bilB = wyt * wx[1] if vB else f32(0.0)
                    else:
                        base = x0 + 1          # x0 == -1: first half IS the x1 tap
                        bilA = wyt * wx[1]
                        bilB = f32(0.0)
                    if bilA == 0.0 and bilB == 0.0:
                        continue
                    rows[q][cam].append(
                        (LV_BASE[lvl] + yy * W + base, lvl, f32(bilA), f32(bilB)))
    return rows


def _balance(rows):
    """Greedy query->core assignment minimizing per-cam max counts."""
    v = np.zeros((Q, N), np.int64)
    for q in range(Q):
        for c in range(N):
            v[q, c] = len(rows[q][c])
    order = np.argsort(-v.sum(1), kind="stable")
    cnt = np.zeros((NCORES, N), np.int64)
    nq = np.zeros(NCORES, np.int64)
    assign = np.zeros(Q, np.int64)
    for q in order:
        mx = cnt.max(0)
        best, bestcost = -1, None
        for ci in range(NCORES):
            if nq[ci] >= QPC:
                continue
            cost = np.maximum(cnt[ci] + v[q] - mx, 0).sum() * 1000 + cnt[ci].sum()
            if bestcost is None or cost < bestcost:
                best, bestcost = ci, cost
        cnt[best] += v[q]
        nq[best] += 1
        assign[q] = best
    qlists = [sorted(np.nonzero(assign == ci)[0].tolist()) for ci in range(NCORES)]
    return qlists, cnt


def _wrap_idx(idx, cap):
    """int16 wrapped layout: logical i -> partition i%16 (replicated x8), col i//16."""
    a = np.asarray(idx, np.int64)
    assert len(a) == cap and cap % 16 == 0
    w = a.reshape(cap // 16, 16).T                        # (16, cap//16)
    return np.tile(w, (8, 1)).astype(np.int16)


def _prep(inputs):
    rows = _geometry(inputs)
    qlists, cnt = _balance(rows)
    caps = [int(min(-(-max(1, int(cnt[:, c].max())) // 16) * 16, 1024))
            if cnt[:, c].max() > 0 else 0 for c in range(N)]
    cams = [c for c in range(N) if caps[c] > 0]
    blocks = []
    for c in cams:
        nb = -(-caps[c] // 128)
        for b in range(nb):
            blocks.append((c, b, min(128, caps[c] - 128 * b)))
    NB = len(blocks)

    feats = [np.asarray(inputs[f"feat{i}"], np.float32) for i in range(4)]
    featT = np.zeros((FEAT_ROWS, C), np.float32)
    for c in range(N):
        for l, (H, W) in enumerate(LVL):
            r0 = c * CAM_ROWS + LV_BASE[l]
            featT[r0:r0 + H * W] = feats[l][0, c].reshape(C, H * W).T

    f32 = np.float32
    W_qe = np.asarray(inputs["W_qe"], f32)
    W_fin = np.asarray(inputs["W_fin"], f32)
    W_out = np.asarray(inputs["W_out"], f32)
    wattn = np.asarray(inputs["W_attn"], f32)
    pw2 = np.asarray(inputs["pe_w2"], f32)
    wof = (W_out @ W_fin).astype(f32)
    bfin_eff = (np.asarray(inputs["b_out"], f32) @ W_fin
                + np.asarray(inputs["b_fin"], f32))

    # pk1: [ln4(1024) | gnbn(128) | pb2r(256) | bfinr(64)]  -> (1, 1472)
    pk1 = np.concatenate([
        np.concatenate([np.asarray(inputs[k], f32)
                        for k in ("pe_g1", "pe_be1", "pe_g2", "pe_be2")]),
        np.asarray(inputs["g_norm"], f32), np.asarray(inputs["b_norm"], f32),
        np.asarray(inputs["pe_b2"], f32), bfin_eff,
    ]).reshape(1, 1472)
    # pk4: [rph placeholder(128) | pw1b(256)] per-core rph filled later
    pw1b = np.vstack([np.asarray(inputs["pe_w1"], f32),
                      np.asarray(inputs["pe_b1"], f32)[None]])     # (4, 256)
    # pkB (128, 768): [pw2_0|pw2_1|wfin0|wfin1|wof0|wof1]
    pkB = np.concatenate([pw2[0:128], pw2[128:256],
                          W_fin[0:128], W_fin[128:256],
                          wof[0:128], wof[128:256]], axis=1)
    wqeb = np.vstack([W_qe, np.asarray(inputs["b_qe"], f32)[None]])  # (65, 256)

    shared = dict(
        featT=featT,
        pk1=np.ascontiguousarray(pk1),
        pkB=np.ascontiguousarray(pkB),
        wqeb=np.ascontiguousarray(wqeb),
    )

    query = np.asarray(inputs["query"], f32)[:, 0, :]
    query_pos = np.asarray(inputs["query_pos"], f32)[:, 0, :]
    rp = np.asarray(inputs["reference_points"], f32)[0]
    battn = np.asarray(inputs["b_attn"], f32)

    idx_off = {}
    off = 0
    for c in cams:
        idx_off[c] = off
        off += caps[c] // 16
    IDXW = off

    in_maps = []
    for ci in range(NCORES):
        ql = qlists[ci]
        d = dict(shared)
        d["pk64"] = np.ascontiguousarray(
            np.concatenate([query[ql].T, query_pos[ql].T], axis=1))   # (64, 256)
        d["pk4"] = np.ascontiguousarray(np.concatenate(
            [np.vstack([rp[ql].T, np.ones((1, QPC), f32)]), pw1b], axis=1))  # (4, 384)
        # pkA (128, 24+24+3*NB): [wattn0|wattn1|qidx|bil]
        qidx = np.full((128, NB), -1.0, f32)
        bil = np.zeros((128, 2 * NB), f32)
        metaL = np.zeros((24, 128 * NB), f32)
        percam = {c: [] for c in cams}
        for qlocal, q in enumerate(ql):
            for c in cams:
                for (idx, lvl, bilA, bilB) in rows[q][c]:
                    percam[c].append((idx, qlocal, lvl, bilA, bilB))
        for j, (c, b, R) in enumerate(blocks):
            lst = percam[c]
            for r in range(R):
                i = 128 * b + r
                if i < len(lst):
                    idx, qlocal, lvl, bilA, bilB = lst[i]
                    qidx[r, j] = qlocal
                    bil[r, 2 * j] = bilA
                    bil[r, 2 * j + 1] = bilB
                    metaL[4 * c + lvl, 128 * j + r] = 1.0
        d["pkA"] = np.ascontiguousarray(
            np.concatenate([wattn[0:128], wattn[128:256], qidx, bil], axis=1))
        d["pk24"] = np.ascontiguousarray(
            np.concatenate([battn.reshape(24, 1), metaL], axis=1))   # (24, 1+128NB)
        pkidx = np.zeros((128, IDXW), np.int16)
        for c in cams:
            lst = percam[c]
            ilist = [e[0] for e in lst] + [0] * (caps[c] - len(lst))
            pkidx[:, idx_off[c]:idx_off[c] + caps[c] // 16] = _wrap_idx(ilist, caps[c])
        d["pkidx"] = np.ascontiguousarray(pkidx)
        in_maps.append(d)
    return in_maps, qlists, tuple(caps)


# ---------------------------------------------------------------- device
def _ap(t, off, nparts, dims):
    """Sub-AP of a pool tile: partition count nparts, custom free dims."""
    import concourse.bass as bass
    return bass.AP(t.tensor, t.offset + off,
                   [[t.ap[0][0], nparts]] + [list(d) for d in dims])


def build_nc(caps):
    import concourse.bass as bass
    import concourse.bacc as bacc
    import concourse.mybir as mybir
    import concourse.tile as tile

    f32 = mybir.dt.float32
    f32r = mybir.dt.float32r
    i16 = mybir.dt.int16
    i32 = mybir.dt.int32
    Alu = mybir.AluOpType
    Act = mybir.ActivationFunctionType

    cams = [c for c in range(N) if caps[c] > 0]
    blocks = []
    for c in cams:
        nb = -(-caps[c] // 128)
        for b in range(nb):
            blocks.append((c, b, min(128, caps[c] - 128 * b)))
    NB = len(blocks)
    idx_off = {}
    off = 0
    for c in cams:
        idx_off[c] = off
        off += caps[c] // 16
    IDXW = off
    CHUNKS = [list(range(s, min(s + 4, NB))) for s in range(0, NB, 4)]

    nc = bacc.Bacc("TRN2", target_bir_lowering=False, debug=False,
                   enable_asserts=False, num_devices=NCORES,
                   num_swdge_queues=4)

    def din(name, shape, dtype=f32):
        return nc.dram_tensor(name, list(shape), dtype, kind="ExternalInput").ap()

    featT = din("featT", (FEAT_ROWS, C))
    pk1_d = din("pk1", (1, 1472))
    pk4_d = din("pk4", (4, 384))
    pk64_d = din("pk64", (64, 256))
    wqeb_d = din("wqeb", (65, 256))
    pkA_d = din("pkA", (128, 48 + 3 * NB))
    pkB_d = din("pkB", (128, 768))
    pk24_d = din("pk24", (24, 1 + 128 * NB))
    pkidx_d = din("pkidx", (128, IDXW), i16)

    out_d = nc.dram_tensor("out", [QPC, 64], f32, kind="ExternalOutput").ap()

    from contextlib import ExitStack
    with tile.TileContext(nc) as tc, ExitStack() as stack:
        cp = stack.enter_context(tc.tile_pool(name="consts", bufs=1))
        wp = stack.enter_context(tc.tile_pool(name="work", bufs=1))
        gp = stack.enter_context(tc.tile_pool(name="gbuf", bufs=1))
        pp = stack.enter_context(tc.tile_pool(name="psum", bufs=4, space="PSUM"))

        V = nc.vector
        S = nc.scalar
        T = nc.tensor
        GS = nc.gpsimd

        def load(dram_ap, shape, name, dtype=f32r, eng=nc.sync):
            t = cp.tile(list(shape), dtype, name=name)
            if dtype in (f32r,):
                dram_ap = dram_ap.bitcast(f32r)
            eng.dma_start(out=t[:, :], in_=dram_ap)
            return t

        # sync ring, in consumption order (gather indices first)
        pkidx_s = load(pkidx_d, (128, IDXW), "pkidx_s", dtype=i16)
        pk4_s = load(pk4_d, (4, 384), "pk4_s")
        pk1_s = load(pk1_d, (1, 1472), "pk1_s")
        pk64_s = load(pk64_d, (64, 256), "pk64_s")
        wqeb_s = load(wqeb_d, (65, 256), "wqeb_s")
        pkA_s = load(pkA_d, (128, 48 + 3 * NB), "pkA_s")
        pk24_s = load(pk24_d, (24, 1 + 128 * NB), "pk24_s")
        pkB_s = load(pkB_d, (128, 768), "pkB_s")
        qidx_c = pkA_s[:, 48:48 + NB]
        bil_c = pkA_s[:, 48 + NB:48 + 3 * NB]

        def vt(shape, name, dtype=f32r, pool=wp, **kw):
            return pool.tile(list(shape), dtype, name=name, **kw)

        # ---------------- on-device iota / identity consts ----------------
        ioti = vt((128, 128), "ioti", dtype=i32)
        GS.iota(ioti[:, :], pattern=[[1, 128]], base=0, channel_multiplier=0)
        iota_f = vt((128, 128), "iota_f", dtype=f32)
        V.tensor_copy(out=iota_f[:, :], in_=ioti[:, :])
        iotc = vt((128, 128), "iotc", dtype=i32)
        GS.iota(iotc[:, :], pattern=[[0, 128]], base=0, channel_multiplier=1)
        ident = vt((128, 128), "ident")
        V.tensor_tensor(out=ident[:, :], in0=ioti[:, :], in1=iotc[:, :],
                        op=Alu.is_equal)
        ones1f = vt((1, 128), "ones1f", dtype=f32)
        V.memset(ones1f[:, :], 1.0)
        ones1t = vt((1, 128), "ones1t")
        V.tensor_copy(out=ones1t[:, :], in_=ones1f[:, :])
        ones1 = ones1t[:, :]

        # ---------------- gathers (launch ASAP) ---------------------------
        g_t = {}
        for c in cams:
            nb = -(-caps[c] // 128)
            t = gp.tile([128, nb * 512], f32r, name=f"g{c}")
            in_ap = bass.AP(featT.tensor, c * CAM_ROWS * C,
                            [[C, CAM_ROWS + 130], [1, 512]]).bitcast(f32r)
            GS.dma_gather(
                out_ap=_ap(t, 0, 128, [[512, nb], [1, 512]]),
                in_ap=in_ap,
                idxs_ap=pkidx_s[:, idx_off[c]:idx_off[c] + caps[c] // 16],
                num_idxs=caps[c], num_idxs_reg=caps[c],
                elem_size=512, elem_step=C, queue_num=len(g_t) % 4)
            g_t[c] = t

        # ---------------- qeT / sigT --------------------------------------
        qs = vt((65, 128), "qs")
        V.tensor_tensor(out=qs[0:64, :], in0=pk64_s[:, 0:128],
                        in1=pk64_s[:, 128:256], op=Alu.add)
        V.tensor_copy(out=qs[64:65, :], in_=ones1f[:, :])
        qs_r = qs[:, :]
        qeT0_p = pp.tile([128, 128], f32, name="qeT0_p", tag="ps")
        T.matmul(qeT0_p[:, :], lhsT=wqeb_s[:, 0:128], rhs=qs_r, start=True, stop=True)
        qeT1_p = pp.tile([128, 128], f32, name="qeT1_p", tag="ps")
        T.matmul(qeT1_p[:, :], lhsT=wqeb_s[:, 128:256], rhs=qs_r, start=True, stop=True)
        qeT0 = vt((128, 128), "qeT0")
        V.tensor_copy(out=qeT0[:, :], in_=qeT0_p[:, :])
        qeT1 = vt((128, 128), "qeT1")
        V.tensor_copy(out=qeT1[:, :], in_=qeT1_p[:, :])
        attwT_p = pp.tile([24, 128], f32, name="attwT_p", tag="ps")
        T.matmul(attwT_p[:, :], lhsT=pkA_s[:, 0:24], rhs=qeT0[:, :], start=True, stop=False)
        T.matmul(attwT_p[:, :], lhsT=pkA_s[:, 24:48], rhs=qeT1[:, :], start=False, stop=True)
        sigT = vt((24, 128), "sigT")
        S.activation(out=sigT[:, :], in_=attwT_p[:, :], func=Act.Sigmoid,
                     bias=pk24_s[:, 0:1])

        # ---------------- fin accumulation chain (qe part, early) ---------
        # pkB cols: [pw2_0(0:256) | pw2_1(256:512) | wfin0(512:576) |
        #            wfin1(576:640) | wof0(640:704) | wof1(704:768)]
        fin_p = pp.tile([128, 64], f32, name="fin_p", tag="fin", bufs=1)
        T.matmul(fin_p[:, :], lhsT=qeT0[:, :], rhs=pkB_s[:, 512:576], start=True, stop=False)
        T.matmul(fin_p[:, :], lhsT=qeT1[:, :], rhs=pkB_s[:, 576:640], start=False, stop=False)

        # ---------------- LN const broadcasts (ones-row matmuls) ----------
        lnbA_p = pp.tile([128, 512], f32, name="lnbA_p", tag="ps")
        T.matmul(lnbA_p[:, :], lhsT=ones1, rhs=pk1_s[:, 0:512], start=True, stop=True)
        lnbB_p = pp.tile([128, 512], f32, name="lnbB_p", tag="ps")
        T.matmul(lnbB_p[:, :], lhsT=ones1, rhs=pk1_s[:, 512:1024], start=True, stop=True)
        gnbn_p = pp.tile([128, 128], f32, name="gnbn_p", tag="ps")
        T.matmul(gnbn_p[:, :], lhsT=ones1, rhs=pk1_s[:, 1024:1152], start=True, stop=True)
        lncA = vt((128, 512), "lncA", dtype=f32)
        V.tensor_copy(out=lncA[:, :], in_=lnbA_p[:, :])
        lncB = vt((128, 512), "lncB", dtype=f32)
        V.tensor_copy(out=lncB[:, :], in_=lnbB_p[:, :])
        gnbn_t = vt((128, 128), "gnbn_t", dtype=f32)
        V.tensor_copy(out=gnbn_t[:, :], in_=gnbn_p[:, :])

        # ---------------- helpers -----------------------------------------
        def transpose2(src, name):
            t0p = pp.tile([128, 128], f32r, name=f"{name}0p", tag="ps")
            T.transpose(t0p[:, :], src[:, 0:128], ident[:, :])
            t1p = pp.tile([128, 128], f32r, name=f"{name}1p", tag="ps")
            T.transpose(t1p[:, :], src[:, 128:256], ident[:, :])
            t0 = vt((128, 128), f"{name}0")
            t1 = vt((128, 128), f"{name}1")
            V.tensor_copy(out=t0[:, :], in_=t0p[:, :])
            V.tensor_copy(out=t1[:, :], in_=t1p[:, :])
            return t0, t1

        def layer_norm(x_ap, g_ap, b_ap, dim, name):
            mu = vt((128, 1), f"{name}_mu", dtype=f32)
            V.tensor_reduce(out=mu[:, :], in_=x_ap, axis=mybir.AxisListType.X, op=Alu.add)
            V.tensor_scalar_mul(out=mu[:, :], in0=mu[:, :], scalar1=1.0 / dim)
            xm = vt((128, dim), f"{name}_xm", dtype=f32)
            V.tensor_scalar(out=xm[:, :], in0=x_ap, scalar1=mu[:, :], scalar2=None,
                            op0=Alu.subtract)
            sq = vt((128, dim), f"{name}_sq", dtype=f32)
            vs = vt((128, 1), f"{name}_vs", dtype=f32)
            V.scalar_tensor_tensor(out=sq[:, :], in0=xm[:, :], scalar=0.0, in1=xm[:, :],
                                   op0=Alu.add, op1=Alu.mult, accum_out=vs[:, :])
            std = vt((128, 1), f"{name}_std", dtype=f32)
            V.tensor_scalar(out=std[:, :], in0=vs[:, :], scalar1=1.0 / dim,
                            scalar2=1e-5, op0=Alu.mult, op1=Alu.add)
            S.activation(out=std[:, :], in_=std[:, :], func=Act.Sqrt)
            rstd = vt((128, 1), f"{name}_rstd", dtype=f32)
            V.reciprocal(out=rstd[:, :], in_=std[:, :])
            o = vt((128, dim), f"{name}_o", dtype=f32)
            V.scalar_tensor_tensor(out=o[:, :], in0=xm[:, :], scalar=rstd[:, :],
                                   in1=g_ap, op0=Alu.mult, op1=Alu.mult)
            V.tensor_tensor(out=o[:, :], in0=o[:, :], in1=b_ap, op=Alu.add)
            return o

        # ---------------- positional branch -------------------------------
        pos1_p = pp.tile([128, 256], f32, name="pos1_p", tag="ps")
        T.matmul(pos1_p[:, :], lhsT=pk4_s[:, 0:128], rhs=pk4_s[:, 128:384],
                 start=True, stop=True)
        l1 = layer_norm(pos1_p[:, :], lncA[:, 0:256], lncA[:, 256:512], 256, "ln1")
        r1a = vt((128, 256), "r1a")
        V.tensor_scalar_max(out=r1a[:, :], in0=l1[:, :], scalar1=0.0)
        rT0, rT1 = transpose2(r1a, "rT")
        pos2_p = pp.tile([128, 256], f32, name="pos2_p", tag="ps")
        T.matmul(pos2_p[:, :], lhsT=rT0[:, :], rhs=pkB_s[:, 0:256], start=True, stop=False)
        T.matmul(pos2_p[:, :], lhsT=rT1[:, :], rhs=pkB_s[:, 256:512], start=False, stop=False)
        T.matmul(pos2_p[:, :], lhsT=ones1, rhs=pk1_s[:, 1152:1408], start=False, stop=True)
        l2 = layer_norm(pos2_p[:, :], lncB[:, 0:256], lncB[:, 256:512], 256, "ln2")
        pos = vt((128, 256), "pos")
        V.tensor_scalar_max(out=pos[:, :], in0=l2[:, :], scalar1=0.0)
        posT0, posT1 = transpose2(pos, "posT")
        T.matmul(fin_p[:, :], lhsT=posT0[:, :], rhs=pkB_s[:, 512:576], start=False, stop=False)
        T.matmul(fin_p[:, :], lhsT=posT1[:, :], rhs=pkB_s[:, 576:640], start=False, stop=False)

        # ---------------- per-chunk scatter-weighted reduce ----------------
        psum_out = pp.tile([128, 256], f32, name="psum_out", tag="psout", bufs=1)
        sigsel_ps = []
        for ck, chunk in enumerate(CHUNKS):
            nb_ck = len(chunk)
            sp = pp.tile([128, 128 * nb_ck], f32, name=f"sigsel{ck}_p",
                         tag=f"sigsel{ck}", bufs=1)
            for jj, j in enumerate(chunk):
                c, b, R = blocks[j]
                T.matmul(sp[:, 128 * jj:128 * jj + 128],
                         lhsT=pk24_s[:, 1 + 128 * j:1 + 128 * j + 128],
                         rhs=sigT[:, :], start=True, stop=True)
            sigsel_ps.append(sp)
        for ck, chunk in enumerate(CHUNKS):
            nb_ck = len(chunk)
            j0 = chunk[0]
            sp = sigsel_ps[ck]
            ohb = vt((128, 256 * nb_ck), f"ohb{ck}", dtype=f32)
            V.tensor_tensor(
                out=ohb[:, :],
                in0=_ap(iota_f, 0, 128, [[0, nb_ck], [0, 2], [1, 128]]),
                in1=_ap(pkA_s, 48 + j0, 128, [[1, nb_ck], [0, 2], [0, 128]]),
                op=Alu.is_equal)
            V.tensor_tensor(
                out=ohb[:, :], in0=ohb[:, :],
                in1=_ap(pkA_s, 48 + NB + 2 * j0, 128, [[2, nb_ck], [1, 2], [0, 128]]),
                op=Alu.mult)
            diag = vt((128, 256 * nb_ck), f"diag{ck}")
            V.tensor_tensor(
                out=diag[:, :], in0=ohb[:, :],
                in1=_ap(sp, 0, 128, [[128, nb_ck], [0, 2], [1, 128]]),
                op=Alu.mult)
            for jj, j in enumerate(chunk):
                c, b, R = blocks[j]
                for h in range(2):
                    T.matmul(psum_out[:, :],
                             lhsT=diag[0:R, 256 * jj + 128 * h:256 * jj + 128 * h + 128],
                             rhs=g_t[c][0:R, 512 * b + 256 * h:512 * b + 256 * h + 256],
                             start=(j == 0 and h == 0), stop=(j == NB - 1 and h == 1))

        # ---------------- tail ---------------------------------------------
        out_acc = vt((128, 256), "out_acc")
        V.tensor_copy(out=out_acc[:, :], in_=psum_out[:, :])
        oT0, oT1 = transpose2(out_acc, "oT")
        T.matmul(fin_p[:, :], lhsT=oT0[:, :], rhs=pkB_s[:, 640:704], start=False, stop=False)
        T.matmul(fin_p[:, :], lhsT=oT1[:, :], rhs=pkB_s[:, 704:768], start=False, stop=False)
        T.matmul(fin_p[:, :], lhsT=ones1, rhs=pk1_s[:, 1408:1472], start=False, stop=True)
        fo = layer_norm(fin_p[:, :], gnbn_t[:, 0:64], gnbn_t[:, 64:128], 64, "ln3")
        nc.sync.dma_start(out=out_d, in_=fo[:, :])

    nc.compile()
    return nc


# ---------------------------------------------------------------- entry
def _ensure_ntff_hook():
    """Register the axon NTFF profiling hook if the image lacks antenv.axon_hooks."""
    import sys
    import types
    try:
        import antenv.axon_hooks  # noqa: F401
        return
    except ImportError:
        pass
    m = types.ModuleType("antenv.axon_hooks")
    _h = [None]
    m.set_axon_ntff_profile_hook = lambda h: _h.__setitem__(0, h)
    m.get_axon_ntff_profile_hook = lambda: _h[0]
    sys.modules["antenv.axon_hooks"] = m
    try:
        import antenv
        antenv.axon_hooks = m
    except ImportError:
        pass
    try:
        from trn_agent_boot.trn_boot import _ntff_profile_via_ctypes
        hook = _ntff_profile_via_ctypes("/opt/axon/libaxon_pjrt.so")
        if hook is not None:
            m.set_axon_ntff_profile_hook(hook)
    except Exception:
        pass


def kernel(**inputs):
    in_maps, qlists, caps = _prep(inputs)
    if _CACHE.get("caps") != caps:
        _CACHE["nc"] = build_nc(caps)
        _CACHE["caps"] = caps
    nc = _CACHE["nc"]
    if _CACHE.get("trace"):
        _ensure_ntff_hook()
    from concourse.bass_utils import run_bass_kernel_spmd
    res = run_bass_kernel_spmd(nc, in_maps, core_ids=list(range(NCORES)),
                               trace=bool(_CACHE.get("trace")),
                               tmpdir=_CACHE.get("tmpdir"))
    _CACHE["last_results"] = res
    out = np.zeros((Q, 64), np.float32)
    for ci in range(NCORES):
        out[qlists[ci]] = res.results[ci]["out"]
    return out.reshape(Q, B, 64).astype(np.float32)
